# revision 14
# baseline (speedup 1.0000x reference)
"""CrossAttention (text+image context, 16 heads) on 8 Trainium2 NeuronCores.

Sharding: data-parallel over batch (16 batches -> 2 per core). No collectives.

Math per batch b (reference):
  q = x @ Wq                      [2048, 1024] -> heads [2048, 16, 64]
  k/v  = ctx_txt @ Wk/Wv          (77 text tokens)
  k2/v2= ctx_img @ Wk_ip/Wv_ip    (256 image tokens)
  out  = softmax(q k^T / 8) v + softmax(q k2^T / 8) v2
  y    = out @ Wo + bo

Device layout choices:
  - All matmul operands bf16 (fp32 psum accumulate). 4x faster PE than fp32.
  - Host pre-transposes x/context so activations arrive as xT [d_model, tok].
  - q is produced transposed (qT [inner, tok]); sim computed keys-on-partitions
    (simT [keys, tok]) so exp output feeds attn@v directly as the stationary
    operand; softmax denominator = extra ones-column appended to v.
  - attn@v output lands queries-on-partitions [128q, 65] so normalization is a
    per-partition scale; the 64-d head outputs are PE-transposed back to
    [inner, tok] for the output projection; bias added via per-partition
    tensor_scalar_add on the transposed output. Host un-transposes.
"""

import numpy as np
import ml_dtypes

import concourse.bass as bass
import concourse.bacc as bacc
import concourse.tile as tile
import concourse.mybir as mybir
from concourse import bass_utils

BF16 = mybir.dt.bfloat16
F32 = mybir.dt.float32
NPBF16 = ml_dtypes.bfloat16

B, N, D = 16, 2048, 1024        # batch, query tokens, model dim (= inner dim)
H, DH = 16, 64                  # heads, head dim
LT, LI, L = 77, 256, 333        # text len, image len, total context len
NCORES = 8
BL = B // NCORES                # 2 batches per core
P = 128
TC = 512                        # query-token chunk (= one PSUM bank of fp32)
NCH = N // TC                   # 4 chunks per batch
NQS = TC // P                   # 4 query sub-tiles of 128 per chunk
SCALE = DH ** -0.5              # 0.125, folded into the exp() activation

TRACE = False
TMPDIR = None
LAST_RESULT = None
_NC = None


def _build_body(tc, io):
    nc = tc.nc
    import contextlib
    ctx = contextlib.ExitStack()
    with ctx:
        # ---------------- persistent pools ----------------
        constp = ctx.enter_context(tc.tile_pool(name="const", bufs=1))
        wq = constp.tile([P, 8, D], BF16, name="wq")
        wo = constp.tile([P, 8, D], BF16, name="wo")
        ident = constp.tile([P, P], BF16, name="ident")
        bot = constp.tile([P, 8], F32, name="bot")
        for kc in range(8):
            nc.sync.dma_start(
                wq[:, kc, :],
                io["wq"].ap()[kc * P:(kc + 1) * P, :],
            )
            nc.sync.dma_start(
                wo[:, kc, :],
                io["wo"].ap()[kc * P:(kc + 1) * P, :],
            )
        nc.sync.dma_start(ident[:], io["ident"].ap())
        nc.sync.dma_start(bot[:], io["boT"].ap())

        kvp = ctx.enter_context(tc.tile_pool(name="kvout", bufs=1))
        kt_txt, kt_img, v_txt, v_img0, v_img1 = [], [], [], [], []
        for b in range(BL):
            kt_txt.append(kvp.tile([P, 8, LT], BF16, name=f"kttxt{b}"))
            kt_img.append(kvp.tile([P, 8, LI], BF16, name=f"ktimg{b}"))
            v_txt.append(kvp.tile([P, H * 65], BF16, name=f"vtxt{b}"))
            v_img0.append(kvp.tile([P, H * 65], BF16, name=f"vimg0{b}"))
            v_img1.append(kvp.tile([P, H * 65], BF16, name=f"vimg1{b}"))

        # ---------------- phase 1: kv projections ----------------
        with (
            tc.tile_pool(name="kvw", bufs=1) as kvwp,
            tc.tile_pool(name="kvps", bufs=2, space="PSUM") as kvpp,
        ):
            wk = kvwp.tile([P, 8, D], BF16, name="wk")
            wv = kvwp.tile([P, 8, D], BF16, name="wv")
            wkip = kvwp.tile([P, 8, D], BF16, name="wkip")
            wvip = kvwp.tile([P, 8, D], BF16, name="wvip")
            for kc in range(8):
                for wt, nm in ((wk, "wk"), (wkip, "wkip"), (wv, "wv"), (wvip, "wvip")):
                    nc.sync.dma_start(
                        wt[:, kc, :], io[nm].ap()[kc * P:(kc + 1) * P, :]
                    )

            for b in range(BL):
                ctxT = kvwp.tile([P, 8, L], BF16, tag="ctxT", name=f"ctxT{b}")
                nc.sync.dma_start(
                    ctxT[:], io["cT"].ap()[b].rearrange("(kc p) l -> p kc l", p=P)
                )
                # kT projections: out [inner-chunk, keys]
                for ic in range(8):
                    ps = kvpp.tile([P, L], F32, tag="kvk")
                    for kc in range(8):
                        nc.tensor.matmul(
                            ps[:, 0:LT],
                            wk[:, kc, ic * P:(ic + 1) * P],
                            ctxT[:, kc, 0:LT],
                            start=(kc == 0), stop=(kc == 7),
                        )
                    for kc in range(8):
                        nc.tensor.matmul(
                            ps[:, LT:L],
                            wkip[:, kc, ic * P:(ic + 1) * P],
                            ctxT[:, kc, LT:L],
                            start=(kc == 0), stop=(kc == 7),
                        )
                    nc.scalar.copy(kt_txt[b][:, ic, :], ps[:, 0:LT])
                    nc.scalar.copy(kt_img[b][:, ic, :], ps[:, LT:L])
                # v projections: out [keys, inner], evacuated into 65-col head
                # blocks (col 64 of each block later memset to 1.0 -> rowsums)
                for (vout, off, tl, wgt) in (
                    (v_txt[b], 0, LT, wv),
                    (v_img0[b], LT, P, wvip),
                    (v_img1[b], LT + P, P, wvip),
                ):
                    for nh in range(2):
                        ps2 = kvpp.tile([P, TC], F32, tag="kvv")
                        for kc in range(8):
                            nc.tensor.matmul(
                                ps2[0:tl, :],
                                ctxT[:, kc, off:off + tl],
                                wgt[:, kc, nh * TC:(nh + 1) * TC],
                                start=(kc == 0), stop=(kc == 7),
                            )
                        vview = vout[:].rearrange("p (h c) -> p h c", c=65)
                        psview = ps2[:].rearrange("p (h c) -> p h c", c=64)
                        nc.vector.tensor_copy(
                            vview[0:tl, nh * 8:(nh + 1) * 8, 0:64],
                            psview[0:tl, :, :],
                        )
                    vview = vout[:].rearrange("p (h c) -> p h c", c=65)
                    nc.vector.memset(vview[0:tl, :, 64:65], 1.0)

        # ---------------- phase 2: q / attention / out projection ----------------
        # PSUM (8 banks total): "sim" rotates 4 single-bank tiles (sim txt,
        # sim img x2, and the chunk-end transposes), "av" holds txt+img
        # accumulators (2 banks), "fx" rotates q/out-projection accumulators.
        with (
            tc.tile_pool(name="xq", bufs=2) as xqp,
            tc.tile_pool(name="big", bufs=2) as bigp,
            tc.tile_pool(name="sm", bufs=3) as smp,
            tc.tile_pool(name="fxp", bufs=2, space="PSUM") as gpsp,
            tc.tile_pool(name="sps", bufs=2, space="PSUM") as spsp,
            tc.tile_pool(name="avs", bufs=2, space="PSUM") as avsp,
        ):
            for b in range(BL):
                xTr = io["xT"].ap()[b].rearrange("(kc p) n -> p kc n", p=P)
                oTr = io["outT"].ap()[b].rearrange("(oc p) n -> p oc n", p=P)
                for chk in range(NCH):
                    tsl = slice(chk * TC, (chk + 1) * TC)
                    xt = xqp.tile([P, 8, TC], BF16, tag="xt")
                    nc.sync.dma_start(xt[:], xTr[:, :, tsl])

                    # q projection, transposed output: qT [inner, tok]
                    qt = bigp.tile([P, 8, TC], BF16, tag="qt")
                    for ic in range(8):
                        ps = gpsp.tile([P, TC], F32, tag="fx")
                        for kc in range(8):
                            nc.tensor.matmul(
                                ps[:],
                                wq[:, kc, ic * P:(ic + 1) * P],
                                xt[:, kc, :],
                                start=(kc == 0), stop=(kc == 7),
                            )
                        nc.scalar.copy(qt[:, ic, :], ps[:])

                    attnA = bigp.tile([P, NQS, H, DH], BF16, tag="attnA")
                    for h in range(H):
                        ic, po = h // 2, (h % 2) * DH
                        qh = qt[po:po + DH, ic, :]            # [64, TC]
                        # simT = k q^T  (keys on partitions)
                        pst = spsp.tile([P, TC], F32, tag="sim")
                        nc.tensor.matmul(
                            pst[0:LT, :], kt_txt[b][po:po + DH, ic, :], qh,
                            start=True, stop=True,
                        )
                        et = smp.tile([P, TC], BF16, tag="et")
                        nc.scalar.activation(
                            et[0:LT, :], pst[0:LT, :],
                            mybir.ActivationFunctionType.Exp, scale=SCALE,
                        )
                        ei = smp.tile([P, 2, TC], BF16, tag="ei")
                        for k2 in range(2):
                            psi = spsp.tile([P, TC], F32, tag="sim")
                            nc.tensor.matmul(
                                psi[:],
                                kt_img[b][po:po + DH, ic, k2 * P:(k2 + 1) * P],
                                qh,
                                start=True, stop=True,
                            )
                            nc.scalar.activation(
                                ei[:, k2, :], psi[:],
                                mybir.ActivationFunctionType.Exp, scale=SCALE,
                            )
                        # attn @ [v | 1]: out [128q, 65] per query sub-tile;
                        # txt group in bank 0, img group in bank 1
                        av = avsp.tile([P, 2, TC], F32, tag="av")
                        for qs in range(NQS):
                            csl = slice(qs * 65, qs * 65 + 65)
                            nc.tensor.matmul(
                                av[:, 0, csl],
                                et[0:LT, qs * P:(qs + 1) * P],
                                v_txt[b][0:LT, h * 65:(h + 1) * 65],
                                start=True, stop=True,
                            )
                            nc.tensor.matmul(
                                av[:, 1, csl],
                                ei[:, 0, qs * P:(qs + 1) * P],
                                v_img0[b][:, h * 65:(h + 1) * 65],
                                start=True, stop=False,
                            )
                            nc.tensor.matmul(
                                av[:, 1, csl],
                                ei[:, 1, qs * P:(qs + 1) * P],
                                v_img1[b][:, h * 65:(h + 1) * 65],
                                start=False, stop=True,
                            )
                        # normalize (per-partition = per-query) and sum txt+img
                        r2 = smp.tile([P, 2, NQS], F32, tag="r2")
                        tt = smp.tile([P, 2, NQS, DH], BF16, tag="tt")
                        for g in range(2):
                            avg = av[:, g, 0:NQS * 65].rearrange(
                                "p (q c) -> p q c", c=65
                            )
                            nc.vector.reciprocal(r2[:, g, :], avg[:, :, 64])
                            nc.vector.tensor_tensor(
                                tt[:, g],
                                avg[:, :, 0:DH],
                                r2[:, g, :].to_broadcast((P, NQS, DH)),
                                op=mybir.AluOpType.mult,
                            )
                        nc.vector.tensor_add(attnA[:, :, h, :], tt[:, 0], tt[:, 1])

                    # transpose attn back to [inner, tok] in 128x128 blocks
                    attnT = bigp.tile([P, 8, TC], BF16, tag="attnT")
                    for qs in range(NQS):
                        for hc in range(8):
                            pt = gpsp.tile([P, P], BF16, tag="fx")
                            nc.tensor.transpose(
                                pt[:], attnA[:, qs, 2 * hc:2 * hc + 2, :], ident[:]
                            )
                            nc.vector.tensor_copy(
                                attnT[:, hc, qs * P:(qs + 1) * P], pt[:]
                            )

                    # output projection (transposed): yT = Wo^T attnT + bo
                    osb = bigp.tile([P, 8, TC], BF16, tag="osb")
                    for oc in range(8):
                        ps = gpsp.tile([P, TC], F32, tag="fx")
                        for ic in range(8):
                            nc.tensor.matmul(
                                ps[:],
                                wo[:, ic, oc * P:(oc + 1) * P],
                                attnT[:, ic, :],
                                start=(ic == 0), stop=(ic == 7),
                            )
                        nc.vector.tensor_scalar_add(
                            osb[:, oc, :], ps[:], bot[:, oc:oc + 1]
                        )
                    nc.sync.dma_start(oTr[:, :, tsl], osb[:])


def build():
    global _NC
    if _NC is not None:
        return _NC
    nc = bacc.Bacc("TRN2", target_bir_lowering=False, debug=False,
                   num_devices=NCORES)
    io = {
        "xT": nc.dram_tensor("xT", [BL, D, N], BF16, kind="ExternalInput"),
        "cT": nc.dram_tensor("cT", [BL, D, L], BF16, kind="ExternalInput"),
        "wq": nc.dram_tensor("wq", [D, D], BF16, kind="ExternalInput"),
        "wk": nc.dram_tensor("wk", [D, D], BF16, kind="ExternalInput"),
        "wv": nc.dram_tensor("wv", [D, D], BF16, kind="ExternalInput"),
        "wkip": nc.dram_tensor("wkip", [D, D], BF16, kind="ExternalInput"),
        "wvip": nc.dram_tensor("wvip", [D, D], BF16, kind="ExternalInput"),
        "wo": nc.dram_tensor("wo", [D, D], BF16, kind="ExternalInput"),
        "boT": nc.dram_tensor("boT", [P, 8], F32, kind="ExternalInput"),
        "ident": nc.dram_tensor("ident", [P, P], BF16, kind="ExternalInput"),
        "outT": nc.dram_tensor("outT", [BL, D, N], BF16, kind="ExternalOutput"),
    }
    with tile.TileContext(nc) as tc:
        _build_body(tc, io)
    nc.compile()
    _NC = nc
    return nc


def kernel(x, context, Wq, Wk, Wv, Wk_ip, Wv_ip, Wo, bo):
    global LAST_RESULT
    nc = build()

    xT = np.ascontiguousarray(x.astype(NPBF16).transpose(0, 2, 1))
    cT = np.ascontiguousarray(context.astype(NPBF16).transpose(0, 2, 1))
    shared = {
        "wq": np.ascontiguousarray(Wq.astype(NPBF16)),
        "wk": np.ascontiguousarray(Wk.astype(NPBF16)),
        "wv": np.ascontiguousarray(Wv.astype(NPBF16)),
        "wkip": np.ascontiguousarray(Wk_ip.astype(NPBF16)),
        "wvip": np.ascontiguousarray(Wv_ip.astype(NPBF16)),
        "wo": np.ascontiguousarray(Wo.astype(NPBF16)),
        "boT": np.ascontiguousarray(bo.astype(np.float32).reshape(8, P).T),
        "ident": np.eye(P, dtype=NPBF16),
    }
    in_maps = [
        {"xT": xT[c * BL:(c + 1) * BL], "cT": cT[c * BL:(c + 1) * BL], **shared}
        for c in range(NCORES)
    ]
    res = bass_utils.run_bass_kernel_spmd(
        nc, in_maps, core_ids=list(range(NCORES)), trace=TRACE, tmpdir=TMPDIR
    )
    LAST_RESULT = res
    out = np.concatenate(
        [r["outT"].transpose(0, 2, 1).astype(np.float32) for r in res.results],
        axis=0,
    )
    return np.ascontiguousarray(out)


# revision 16
# speedup vs baseline: 1.1521x; 1.1521x over previous
"""CrossAttention (text+image context, 16 heads) on 8 Trainium2 NeuronCores.

Sharding: data-parallel over batch (16 batches -> 2 per core). No collectives.

Math per batch b (reference):
  q = x @ Wq                      [2048, 1024] -> heads [2048, 16, 64]
  k/v  = ctx_txt @ Wk/Wv          (77 text tokens)
  k2/v2= ctx_img @ Wk_ip/Wv_ip    (256 image tokens)
  out  = softmax(q k^T / 8) v + softmax(q k2^T / 8) v2
  y    = out @ Wo + bo

Device layout choices:
  - All matmul operands bf16 (fp32 psum accumulate). 4x faster PE than fp32.
  - Host pre-transposes x/context so activations arrive as xT [d_model, tok].
  - q is produced transposed (qT [inner, tok]); sim computed keys-on-partitions
    (simT [keys, tok]) so exp output feeds attn@v directly as the stationary
    operand; softmax denominator = extra ones-column appended to v.
  - attn@v output lands queries-on-partitions [128q, 65] so normalization is a
    per-partition scale; the 64-d head outputs are PE-transposed back to
    [inner, tok] for the output projection; bias added via per-partition
    tensor_scalar_add on the transposed output. Host un-transposes.
"""

import numpy as np
import ml_dtypes

import concourse.bass as bass
import concourse.bacc as bacc
import concourse.tile as tile
import concourse.mybir as mybir
from concourse import bass_utils

BF16 = mybir.dt.bfloat16
F32 = mybir.dt.float32
NPBF16 = ml_dtypes.bfloat16

B, N, D = 16, 2048, 1024        # batch, query tokens, model dim (= inner dim)
H, DH = 16, 64                  # heads, head dim
LT, LI, L = 77, 256, 333        # text len, image len, total context len
NCORES = 8
BL = B // NCORES                # 2 batches per core
P = 128
TC = 512                        # query-token chunk (= one PSUM bank of fp32)
NCH = N // TC                   # 4 chunks per batch
NQS = TC // P                   # 4 query sub-tiles of 128 per chunk
SCALE = DH ** -0.5              # 0.125, folded into the exp() activation

TRACE = False
TMPDIR = None
LAST_RESULT = None
_NC = None


def _build_body(tc, io):
    nc = tc.nc
    import contextlib
    ctx = contextlib.ExitStack()
    with ctx:
        # ---------------- persistent pools ----------------
        constp = ctx.enter_context(tc.tile_pool(name="const", bufs=1))
        wq = constp.tile([P, 8, D], BF16, name="wq")
        wo = constp.tile([P, 8, D], BF16, name="wo")
        ident = constp.tile([P, P], BF16, name="ident")
        bot = constp.tile([P, 8], F32, name="bot")
        for kc in range(8):
            nc.sync.dma_start(
                wq[:, kc, :],
                io["wq"].ap()[kc * P:(kc + 1) * P, :],
            )
            nc.sync.dma_start(
                wo[:, kc, :],
                io["wo"].ap()[kc * P:(kc + 1) * P, :],
            )
        nc.sync.dma_start(ident[:], io["ident"].ap())
        nc.sync.dma_start(bot[:], io["boT"].ap())

        kvp = ctx.enter_context(tc.tile_pool(name="kvout", bufs=1))
        kt_txt, kt_img, v_txt, v_img0, v_img1 = [], [], [], [], []
        for b in range(BL):
            kt_txt.append(kvp.tile([P, 8, LT], BF16, name=f"kttxt{b}"))
            kt_img.append(kvp.tile([P, 8, LI], BF16, name=f"ktimg{b}"))
            v_txt.append(kvp.tile([P, H * 65], BF16, name=f"vtxt{b}"))
            v_img0.append(kvp.tile([P, H * 65], BF16, name=f"vimg0{b}"))
            v_img1.append(kvp.tile([P, H * 65], BF16, name=f"vimg1{b}"))

        # ---------------- phase 1: kv projections ----------------
        with (
            tc.tile_pool(name="kvw", bufs=1) as kvwp,
            tc.tile_pool(name="kvps", bufs=2, space="PSUM") as kvpp,
        ):
            wk = kvwp.tile([P, 8, D], BF16, name="wk")
            wv = kvwp.tile([P, 8, D], BF16, name="wv")
            wkip = kvwp.tile([P, 8, D], BF16, name="wkip")
            wvip = kvwp.tile([P, 8, D], BF16, name="wvip")
            for kc in range(8):
                for wt, nm in ((wk, "wk"), (wkip, "wkip"), (wv, "wv"), (wvip, "wvip")):
                    nc.sync.dma_start(
                        wt[:, kc, :], io[nm].ap()[kc * P:(kc + 1) * P, :]
                    )

            for b in range(BL):
                ctxT = kvwp.tile([P, 8, L], BF16, tag="ctxT", name=f"ctxT{b}")
                nc.sync.dma_start(
                    ctxT[:], io["cT"].ap()[b].rearrange("(kc p) l -> p kc l", p=P)
                )
                # kT projections: out [inner-chunk, keys]
                for ic in range(8):
                    ps = kvpp.tile([P, L], F32, tag="kvk")
                    for kc in range(8):
                        nc.tensor.matmul(
                            ps[:, 0:LT],
                            wk[:, kc, ic * P:(ic + 1) * P],
                            ctxT[:, kc, 0:LT],
                            start=(kc == 0), stop=(kc == 7),
                        )
                    for kc in range(8):
                        nc.tensor.matmul(
                            ps[:, LT:L],
                            wkip[:, kc, ic * P:(ic + 1) * P],
                            ctxT[:, kc, LT:L],
                            start=(kc == 0), stop=(kc == 7),
                        )
                    nc.scalar.copy(kt_txt[b][:, ic, :], ps[:, 0:LT])
                    nc.scalar.copy(kt_img[b][:, ic, :], ps[:, LT:L])
                # v projections: out [keys, inner], evacuated into 65-col head
                # blocks (col 64 of each block later memset to 1.0 -> rowsums)
                for (vout, off, tl, wgt) in (
                    (v_txt[b], 0, LT, wv),
                    (v_img0[b], LT, P, wvip),
                    (v_img1[b], LT + P, P, wvip),
                ):
                    for nh in range(2):
                        ps2 = kvpp.tile([P, TC], F32, tag="kvv")
                        for kc in range(8):
                            nc.tensor.matmul(
                                ps2[0:tl, :],
                                ctxT[:, kc, off:off + tl],
                                wgt[:, kc, nh * TC:(nh + 1) * TC],
                                start=(kc == 0), stop=(kc == 7),
                            )
                        vview = vout[:].rearrange("p (h c) -> p h c", c=65)
                        psview = ps2[:].rearrange("p (h c) -> p h c", c=64)
                        nc.vector.tensor_copy(
                            vview[0:tl, nh * 8:(nh + 1) * 8, 0:64],
                            psview[0:tl, :, :],
                        )
                    vview = vout[:].rearrange("p (h c) -> p h c", c=65)
                    nc.vector.memset(vview[0:tl, :, 64:65], 1.0)

        # ---------------- phase 2: q / attention / out projection ----------------
        # PSUM (8 banks total): "sim" rotates 4 single-bank tiles (sim txt,
        # sim img x2, and the chunk-end transposes), "av" holds txt+img
        # accumulators (2 banks), "fx" rotates q/out-projection accumulators.
        with (
            tc.tile_pool(name="xq", bufs=2) as xqp,
            tc.tile_pool(name="big", bufs=2) as bigp,
            tc.tile_pool(name="sm", bufs=3) as smp,
            tc.tile_pool(name="fxp", bufs=2, space="PSUM") as gpsp,
            tc.tile_pool(name="sps", bufs=4, space="PSUM") as spsp,
            tc.tile_pool(name="avs", bufs=1, space="PSUM") as avsp,
        ):
            for b in range(BL):
                xTr = io["xT"].ap()[b].rearrange("(kc p) n -> p kc n", p=P)
                oTr = io["outT"].ap()[b].rearrange("(oc p) n -> p oc n", p=P)
                for chk in range(NCH):
                    tsl = slice(chk * TC, (chk + 1) * TC)
                    xt = xqp.tile([P, 8, TC], BF16, tag="xt")
                    nc.sync.dma_start(xt[:], xTr[:, :, tsl])

                    # q projection, transposed output: qT [inner, tok]
                    qt = bigp.tile([P, 8, TC], BF16, tag="qt")
                    for ic in range(8):
                        ps = gpsp.tile([P, TC], F32, tag="fx")
                        for kc in range(8):
                            nc.tensor.matmul(
                                ps[:],
                                wq[:, kc, ic * P:(ic + 1) * P],
                                xt[:, kc, :],
                                start=(kc == 0), stop=(kc == 7),
                            )
                        nc.scalar.copy(qt[:, ic, :], ps[:])

                    attnA = bigp.tile([P, NQS, H, DH], BF16, tag="attnA")
                    for h in range(H):
                        ic, po = h // 2, (h % 2) * DH
                        qh = qt[po:po + DH, ic, :]            # [64, TC]
                        # simT = k q^T  (keys on partitions)
                        pst = spsp.tile([P, TC], F32, tag="sim")
                        nc.tensor.matmul(
                            pst[0:LT, :], kt_txt[b][po:po + DH, ic, :], qh,
                            start=True, stop=True,
                        )
                        et = smp.tile([P, TC], BF16, tag="et")
                        nc.scalar.activation(
                            et[0:LT, :], pst[0:LT, :],
                            mybir.ActivationFunctionType.Exp, scale=SCALE,
                        )
                        ei = smp.tile([P, 2, TC], BF16, tag="ei")
                        for k2 in range(2):
                            psi = spsp.tile([P, TC], F32, tag="sim")
                            nc.tensor.matmul(
                                psi[:],
                                kt_img[b][po:po + DH, ic, k2 * P:(k2 + 1) * P],
                                qh,
                                start=True, stop=True,
                            )
                            nc.scalar.activation(
                                ei[:, k2, :], psi[:],
                                mybir.ActivationFunctionType.Exp, scale=SCALE,
                            )
                        # attn @ [v | 1]: out [128q, 65] per query sub-tile;
                        # txt group in bank 0, img group in bank 1
                        av = avsp.tile([P, 2, TC], F32, tag="av")
                        for qs in range(NQS):
                            csl = slice(qs * 65, qs * 65 + 65)
                            nc.tensor.matmul(
                                av[:, 0, csl],
                                et[0:LT, qs * P:(qs + 1) * P],
                                v_txt[b][0:LT, h * 65:(h + 1) * 65],
                                start=True, stop=True,
                            )
                            nc.tensor.matmul(
                                av[:, 1, csl],
                                ei[:, 0, qs * P:(qs + 1) * P],
                                v_img0[b][:, h * 65:(h + 1) * 65],
                                start=True, stop=False,
                            )
                            nc.tensor.matmul(
                                av[:, 1, csl],
                                ei[:, 1, qs * P:(qs + 1) * P],
                                v_img1[b][:, h * 65:(h + 1) * 65],
                                start=False, stop=True,
                            )
                        # normalize (per-partition = per-query) and sum txt+img
                        r2 = smp.tile([P, 2, NQS], F32, tag="r2")
                        tt = smp.tile([P, 2, NQS, DH], BF16, tag="tt")
                        for g in range(2):
                            avg = av[:, g, 0:NQS * 65].rearrange(
                                "p (q c) -> p q c", c=65
                            )
                            nc.vector.reciprocal(r2[:, g, :], avg[:, :, 64])
                            nc.vector.tensor_tensor(
                                tt[:, g],
                                avg[:, :, 0:DH],
                                r2[:, g, :].to_broadcast((P, NQS, DH)),
                                op=mybir.AluOpType.mult,
                            )
                        nc.vector.tensor_add(attnA[:, :, h, :], tt[:, 0], tt[:, 1])

                    # transpose attn back to [inner, tok] in 128x128 blocks
                    attnT = bigp.tile([P, 8, TC], BF16, tag="attnT")
                    for qs in range(NQS):
                        for hc in range(8):
                            pt = spsp.tile([P, P], BF16, tag="sim")
                            nc.tensor.transpose(
                                pt[:], attnA[:, qs, 2 * hc:2 * hc + 2, :], ident[:]
                            )
                            nc.vector.tensor_copy(
                                attnT[:, hc, qs * P:(qs + 1) * P], pt[:]
                            )

                    # output projection (transposed): yT = Wo^T attnT + bo
                    osb = bigp.tile([P, 8, TC], BF16, tag="osb")
                    for oc in range(8):
                        ps = gpsp.tile([P, TC], F32, tag="fx")
                        for ic in range(8):
                            nc.tensor.matmul(
                                ps[:],
                                wo[:, ic, oc * P:(oc + 1) * P],
                                attnT[:, ic, :],
                                start=(ic == 0), stop=(ic == 7),
                            )
                        nc.vector.tensor_scalar_add(
                            osb[:, oc, :], ps[:], bot[:, oc:oc + 1]
                        )
                    nc.sync.dma_start(oTr[:, :, tsl], osb[:])


def build():
    global _NC
    if _NC is not None:
        return _NC
    nc = bacc.Bacc("TRN2", target_bir_lowering=False, debug=False,
                   num_devices=NCORES)
    io = {
        "xT": nc.dram_tensor("xT", [BL, D, N], BF16, kind="ExternalInput"),
        "cT": nc.dram_tensor("cT", [BL, D, L], BF16, kind="ExternalInput"),
        "wq": nc.dram_tensor("wq", [D, D], BF16, kind="ExternalInput"),
        "wk": nc.dram_tensor("wk", [D, D], BF16, kind="ExternalInput"),
        "wv": nc.dram_tensor("wv", [D, D], BF16, kind="ExternalInput"),
        "wkip": nc.dram_tensor("wkip", [D, D], BF16, kind="ExternalInput"),
        "wvip": nc.dram_tensor("wvip", [D, D], BF16, kind="ExternalInput"),
        "wo": nc.dram_tensor("wo", [D, D], BF16, kind="ExternalInput"),
        "boT": nc.dram_tensor("boT", [P, 8], F32, kind="ExternalInput"),
        "ident": nc.dram_tensor("ident", [P, P], BF16, kind="ExternalInput"),
        "outT": nc.dram_tensor("outT", [BL, D, N], BF16, kind="ExternalOutput"),
    }
    with tile.TileContext(nc) as tc:
        _build_body(tc, io)
    nc.compile()
    _NC = nc
    return nc


def kernel(x, context, Wq, Wk, Wv, Wk_ip, Wv_ip, Wo, bo):
    global LAST_RESULT
    nc = build()

    xT = np.ascontiguousarray(x.astype(NPBF16).transpose(0, 2, 1))
    cT = np.ascontiguousarray(context.astype(NPBF16).transpose(0, 2, 1))
    shared = {
        "wq": np.ascontiguousarray(Wq.astype(NPBF16)),
        "wk": np.ascontiguousarray(Wk.astype(NPBF16)),
        "wv": np.ascontiguousarray(Wv.astype(NPBF16)),
        "wkip": np.ascontiguousarray(Wk_ip.astype(NPBF16)),
        "wvip": np.ascontiguousarray(Wv_ip.astype(NPBF16)),
        "wo": np.ascontiguousarray(Wo.astype(NPBF16)),
        "boT": np.ascontiguousarray(bo.astype(np.float32).reshape(8, P).T),
        "ident": np.eye(P, dtype=NPBF16),
    }
    in_maps = [
        {"xT": xT[c * BL:(c + 1) * BL], "cT": cT[c * BL:(c + 1) * BL], **shared}
        for c in range(NCORES)
    ]
    res = bass_utils.run_bass_kernel_spmd(
        nc, in_maps, core_ids=list(range(NCORES)), trace=TRACE, tmpdir=TMPDIR
    )
    LAST_RESULT = res
    out = np.concatenate(
        [r["outT"].transpose(0, 2, 1).astype(np.float32) for r in res.results],
        axis=0,
    )
    return np.ascontiguousarray(out)


# revision 19
# speedup vs baseline: 1.2141x; 1.0538x over previous
"""CrossAttention (text+image context, 16 heads) on 8 Trainium2 NeuronCores.

Sharding: data-parallel over batch (16 batches -> 2 per core). No collectives.

Math per batch b (reference):
  q = x @ Wq                      [2048, 1024] -> heads [2048, 16, 64]
  k/v  = ctx_txt @ Wk/Wv          (77 text tokens)
  k2/v2= ctx_img @ Wk_ip/Wv_ip    (256 image tokens)
  out  = softmax(q k^T / 8) v + softmax(q k2^T / 8) v2
  y    = out @ Wo + bo

Device layout choices:
  - All matmul operands bf16 (fp32 psum accumulate). 4x faster PE than fp32.
  - Host pre-transposes x/context so activations arrive as xT [d_model, tok].
  - q is produced transposed (qT [inner, tok]); sim computed keys-on-partitions
    (simT [keys, tok]) so exp output feeds attn@v directly as the stationary
    operand; softmax denominator = extra ones-column appended to v.
  - attn@v output lands queries-on-partitions [128q, 65] so normalization is a
    per-partition scale; the 64-d head outputs are PE-transposed back to
    [inner, tok] for the output projection; bias added via per-partition
    tensor_scalar_add on the transposed output. Host un-transposes.
"""

import numpy as np
import ml_dtypes

import concourse.bass as bass
import concourse.bacc as bacc
import concourse.tile as tile
import concourse.mybir as mybir
from concourse import bass_utils

BF16 = mybir.dt.bfloat16
F32 = mybir.dt.float32
NPBF16 = ml_dtypes.bfloat16

B, N, D = 16, 2048, 1024        # batch, query tokens, model dim (= inner dim)
H, DH = 16, 64                  # heads, head dim
LT, LI, L = 77, 256, 333        # text len, image len, total context len
NCORES = 8
BL = B // NCORES                # 2 batches per core
P = 128
TC = 512                        # query-token chunk (= one PSUM bank of fp32)
NCH = N // TC                   # 4 chunks per batch
NQS = TC // P                   # 4 query sub-tiles of 128 per chunk
SCALE = DH ** -0.5              # 0.125, folded into the exp() activation

TRACE = False
TMPDIR = None
LAST_RESULT = None
_NC = None


def _build_body(tc, io):
    nc = tc.nc
    import contextlib
    ctx = contextlib.ExitStack()
    with ctx:
        # ---------------- persistent pools ----------------
        constp = ctx.enter_context(tc.tile_pool(name="const", bufs=1))
        wq = constp.tile([P, 8, D], BF16, name="wq")
        wo = constp.tile([P, 8, D], BF16, name="wo")
        ident = constp.tile([P, P], BF16, name="ident")
        bot = constp.tile([P, 8], F32, name="bot")
        nc.sync.dma_start(ident[:], io["ident"].ap())
        nc.sync.dma_start(bot[:], io["boT"].ap())

        kvp = ctx.enter_context(tc.tile_pool(name="kvout", bufs=1))
        kt_txt, kt_img, v_txt, v_img0, v_img1 = [], [], [], [], []
        for b in range(BL):
            kt_txt.append(kvp.tile([P, 8, LT], BF16, name=f"kttxt{b}"))
            kt_img.append(kvp.tile([P, 8, LI], BF16, name=f"ktimg{b}"))
            v_txt.append(kvp.tile([P, H * 65], BF16, name=f"vtxt{b}"))
            v_img0.append(kvp.tile([P, H * 65], BF16, name=f"vimg0{b}"))
            v_img1.append(kvp.tile([P, H * 65], BF16, name=f"vimg1{b}"))

        # ---------------- phase 2 pools opened early (xt prefetch) --------
        xqp = ctx.enter_context(tc.tile_pool(name="xq", bufs=2))
        xTrs = [io["xT"].ap()[b].rearrange("(kc p) n -> p kc n", p=P)
                for b in range(BL)]
        xts = {}

        def load_xt(b, ch):
            t = xqp.tile([P, 8, TC], BF16, tag="xt", name=f"xt{b}_{ch}")
            nc.sync.dma_start(t[:], xTrs[b][:, :, ch * TC:(ch + 1) * TC])
            xts[(b, ch)] = t

        # ---------------- phase 1: kv projections ----------------
        # DMA issue order matters (single sync queue): context + kv weights
        # first, then the first x chunk, then Wq; Wo (needed last) at the end.
        with (
            tc.tile_pool(name="kvw", bufs=1) as kvwp,
            tc.tile_pool(name="kvps", bufs=2, space="PSUM") as kvpp,
        ):
            wk = kvwp.tile([P, 8, D], BF16, name="wk")
            wv = kvwp.tile([P, 8, D], BF16, name="wv")
            wkip = kvwp.tile([P, 8, D], BF16, name="wkip")
            wvip = kvwp.tile([P, 8, D], BF16, name="wvip")
            ctxTs = []
            for b in range(BL):
                ctxTs.append(
                    kvwp.tile([P, 8, L], BF16, tag="ctxT", name=f"ctxT{b}", bufs=2)
                )
                nc.sync.dma_start(
                    ctxTs[b][:], io["cT"].ap()[b].rearrange("(kc p) l -> p kc l", p=P)
                )
            for kc in range(8):
                for wt, nm in ((wk, "wk"), (wkip, "wkip")):
                    nc.sync.dma_start(
                        wt[:, kc, :], io[nm].ap()[kc * P:(kc + 1) * P, :]
                    )
            for kc in range(8):
                for wt, nm in ((wv, "wv"), (wvip, "wvip")):
                    nc.sync.dma_start(
                        wt[:, kc, :], io[nm].ap()[kc * P:(kc + 1) * P, :]
                    )
            load_xt(0, 0)
            for kc in range(8):
                nc.sync.dma_start(wq[:, kc, :], io["wq"].ap()[kc * P:(kc + 1) * P, :])
            nc.sync.dma_start(ident[:], io["ident"].ap())
            nc.sync.dma_start(bot[:], io["boT"].ap())
            for kc in range(8):
                nc.sync.dma_start(wo[:, kc, :], io["wo"].ap()[kc * P:(kc + 1) * P, :])

            for b in range(BL):
                ctxT = ctxTs[b]
                # kT projections: out [inner-chunk, keys]
                for ic in range(8):
                    ps = kvpp.tile([P, L], F32, tag="kvk")
                    for kc in range(8):
                        nc.tensor.matmul(
                            ps[:, 0:LT],
                            wk[:, kc, ic * P:(ic + 1) * P],
                            ctxT[:, kc, 0:LT],
                            start=(kc == 0), stop=(kc == 7),
                        )
                    for kc in range(8):
                        nc.tensor.matmul(
                            ps[:, LT:L],
                            wkip[:, kc, ic * P:(ic + 1) * P],
                            ctxT[:, kc, LT:L],
                            start=(kc == 0), stop=(kc == 7),
                        )
                    nc.scalar.copy(kt_txt[b][:, ic, :], ps[:, 0:LT])
                    nc.scalar.copy(kt_img[b][:, ic, :], ps[:, LT:L])
                # v projections: out [keys, inner], evacuated into 65-col head
                # blocks (col 64 of each block later memset to 1.0 -> rowsums)
                for (vout, off, tl, wgt) in (
                    (v_txt[b], 0, LT, wv),
                    (v_img0[b], LT, P, wvip),
                    (v_img1[b], LT + P, P, wvip),
                ):
                    for nh in range(2):
                        ps2 = kvpp.tile([P, TC], F32, tag="kvv")
                        for kc in range(8):
                            nc.tensor.matmul(
                                ps2[0:tl, :],
                                ctxT[:, kc, off:off + tl],
                                wgt[:, kc, nh * TC:(nh + 1) * TC],
                                start=(kc == 0), stop=(kc == 7),
                            )
                        vview = vout[:].rearrange("p (h c) -> p h c", c=65)
                        psview = ps2[:].rearrange("p (h c) -> p h c", c=64)
                        nc.vector.tensor_copy(
                            vview[0:tl, nh * 8:(nh + 1) * 8, 0:64],
                            psview[0:tl, :, :],
                        )
                    vview = vout[:].rearrange("p (h c) -> p h c", c=65)
                    nc.vector.memset(vview[0:tl, :, 64:65], 1.0)

        # ---------------- phase 2: q / attention / out projection ----------------
        # PSUM (8 banks total): "sim" rotates 4 single-bank tiles (sim txt,
        # sim img x2, and the chunk-end transposes), "av" holds txt+img
        # accumulators (2 banks), "fx" rotates q/out-projection accumulators.
        with (
            tc.tile_pool(name="big", bufs=2) as bigp,
            tc.tile_pool(name="sm", bufs=3) as smp,
            tc.tile_pool(name="fxp", bufs=2, space="PSUM") as gpsp,
            tc.tile_pool(name="sps", bufs=4, space="PSUM") as spsp,
            tc.tile_pool(name="avs", bufs=1, space="PSUM") as avsp,
        ):
            oTrs = [io["outT"].ap()[b].rearrange("(oc p) n -> p oc n", p=P)
                    for b in range(BL)]
            pairs = [(b, ch) for b in range(BL) for ch in range(NCH)]
            for pi, (b, chk) in enumerate(pairs):
                    oTr = oTrs[b]
                    tsl = slice(chk * TC, (chk + 1) * TC)
                    if pi + 1 < len(pairs):
                        load_xt(*pairs[pi + 1])
                    xt = xts.pop((b, chk))

                    # q projection, transposed output: qT [inner, tok]
                    qt = bigp.tile([P, 8, TC], BF16, tag="qt")
                    for ic in range(8):
                        ps = gpsp.tile([P, TC], F32, tag="fx")
                        for kc in range(8):
                            nc.tensor.matmul(
                                ps[:],
                                wq[:, kc, ic * P:(ic + 1) * P],
                                xt[:, kc, :],
                                start=(kc == 0), stop=(kc == 7),
                            )
                        nc.scalar.copy(qt[:, ic, :], ps[:])

                    attnA = bigp.tile([P, NQS, H, DH], BF16, tag="attnA")
                    for h in range(H):
                        ic, po = h // 2, (h % 2) * DH
                        qh = qt[po:po + DH, ic, :]            # [64, TC]
                        # simT = k q^T  (keys on partitions)
                        pst = spsp.tile([P, TC], F32, tag="sim")
                        nc.tensor.matmul(
                            pst[0:LT, :], kt_txt[b][po:po + DH, ic, :], qh,
                            start=True, stop=True,
                        )
                        et = smp.tile([P, TC], BF16, tag="et")
                        nc.scalar.activation(
                            et[0:LT, :], pst[0:LT, :],
                            mybir.ActivationFunctionType.Exp, scale=SCALE,
                        )
                        ei = smp.tile([P, 2, TC], BF16, tag="ei")
                        for k2 in range(2):
                            psi = spsp.tile([P, TC], F32, tag="sim")
                            nc.tensor.matmul(
                                psi[:],
                                kt_img[b][po:po + DH, ic, k2 * P:(k2 + 1) * P],
                                qh,
                                start=True, stop=True,
                            )
                            nc.scalar.activation(
                                ei[:, k2, :], psi[:],
                                mybir.ActivationFunctionType.Exp, scale=SCALE,
                            )
                        # attn @ [v | 1]: out [128q, 65] per query sub-tile;
                        # txt group in bank 0, img group in bank 1
                        av = avsp.tile([P, 2, TC], F32, tag="av")
                        for qs in range(NQS):
                            csl = slice(qs * 65, qs * 65 + 65)
                            nc.tensor.matmul(
                                av[:, 0, csl],
                                et[0:LT, qs * P:(qs + 1) * P],
                                v_txt[b][0:LT, h * 65:(h + 1) * 65],
                                start=True, stop=True,
                            )
                            nc.tensor.matmul(
                                av[:, 1, csl],
                                ei[:, 0, qs * P:(qs + 1) * P],
                                v_img0[b][:, h * 65:(h + 1) * 65],
                                start=True, stop=False,
                            )
                            nc.tensor.matmul(
                                av[:, 1, csl],
                                ei[:, 1, qs * P:(qs + 1) * P],
                                v_img1[b][:, h * 65:(h + 1) * 65],
                                start=False, stop=True,
                            )
                        # normalize (per-partition = per-query) and sum txt+img
                        r2 = smp.tile([P, 2, NQS], F32, tag="r2")
                        tt = smp.tile([P, 2, NQS, DH], BF16, tag="tt")
                        for g in range(2):
                            avg = av[:, g, 0:NQS * 65].rearrange(
                                "p (q c) -> p q c", c=65
                            )
                            nc.vector.reciprocal(r2[:, g, :], avg[:, :, 64])
                            nc.vector.tensor_tensor(
                                tt[:, g],
                                avg[:, :, 0:DH],
                                r2[:, g, :].to_broadcast((P, NQS, DH)),
                                op=mybir.AluOpType.mult,
                            )
                        nc.vector.tensor_add(attnA[:, :, h, :], tt[:, 0], tt[:, 1])

                    # transpose attn back to [inner, tok] in 128x128 blocks
                    attnT = bigp.tile([P, 8, TC], BF16, tag="attnT")
                    for qs in range(NQS):
                        for hc in range(8):
                            pt = spsp.tile([P, P], BF16, tag="sim")
                            nc.tensor.transpose(
                                pt[:], attnA[:, qs, 2 * hc:2 * hc + 2, :], ident[:]
                            )
                            nc.vector.tensor_copy(
                                attnT[:, hc, qs * P:(qs + 1) * P], pt[:]
                            )

                    # output projection (transposed): yT = Wo^T attnT + bo
                    osb = bigp.tile([P, 8, TC], BF16, tag="osb")
                    for oc in range(8):
                        ps = gpsp.tile([P, TC], F32, tag="fx")
                        for ic in range(8):
                            nc.tensor.matmul(
                                ps[:],
                                wo[:, ic, oc * P:(oc + 1) * P],
                                attnT[:, ic, :],
                                start=(ic == 0), stop=(ic == 7),
                            )
                        nc.vector.tensor_scalar_add(
                            osb[:, oc, :], ps[:], bot[:, oc:oc + 1]
                        )
                    nc.sync.dma_start(oTr[:, :, tsl], osb[:])


def build():
    global _NC
    if _NC is not None:
        return _NC
    nc = bacc.Bacc("TRN2", target_bir_lowering=False, debug=False,
                   num_devices=NCORES)
    io = {
        "xT": nc.dram_tensor("xT", [BL, D, N], BF16, kind="ExternalInput"),
        "cT": nc.dram_tensor("cT", [BL, D, L], BF16, kind="ExternalInput"),
        "wq": nc.dram_tensor("wq", [D, D], BF16, kind="ExternalInput"),
        "wk": nc.dram_tensor("wk", [D, D], BF16, kind="ExternalInput"),
        "wv": nc.dram_tensor("wv", [D, D], BF16, kind="ExternalInput"),
        "wkip": nc.dram_tensor("wkip", [D, D], BF16, kind="ExternalInput"),
        "wvip": nc.dram_tensor("wvip", [D, D], BF16, kind="ExternalInput"),
        "wo": nc.dram_tensor("wo", [D, D], BF16, kind="ExternalInput"),
        "boT": nc.dram_tensor("boT", [P, 8], F32, kind="ExternalInput"),
        "ident": nc.dram_tensor("ident", [P, P], BF16, kind="ExternalInput"),
        "outT": nc.dram_tensor("outT", [BL, D, N], BF16, kind="ExternalOutput"),
    }
    with tile.TileContext(nc) as tc:
        _build_body(tc, io)
    nc.compile()
    _NC = nc
    return nc


def kernel(x, context, Wq, Wk, Wv, Wk_ip, Wv_ip, Wo, bo):
    global LAST_RESULT
    nc = build()

    xT = np.ascontiguousarray(x.astype(NPBF16).transpose(0, 2, 1))
    cT = np.ascontiguousarray(context.astype(NPBF16).transpose(0, 2, 1))
    shared = {
        "wq": np.ascontiguousarray(Wq.astype(NPBF16)),
        "wk": np.ascontiguousarray(Wk.astype(NPBF16)),
        "wv": np.ascontiguousarray(Wv.astype(NPBF16)),
        "wkip": np.ascontiguousarray(Wk_ip.astype(NPBF16)),
        "wvip": np.ascontiguousarray(Wv_ip.astype(NPBF16)),
        "wo": np.ascontiguousarray(Wo.astype(NPBF16)),
        "boT": np.ascontiguousarray(bo.astype(np.float32).reshape(8, P).T),
        "ident": np.eye(P, dtype=NPBF16),
    }
    in_maps = [
        {"xT": xT[c * BL:(c + 1) * BL], "cT": cT[c * BL:(c + 1) * BL], **shared}
        for c in range(NCORES)
    ]
    res = bass_utils.run_bass_kernel_spmd(
        nc, in_maps, core_ids=list(range(NCORES)), trace=TRACE, tmpdir=TMPDIR
    )
    LAST_RESULT = res
    out = np.concatenate(
        [r["outT"].transpose(0, 2, 1).astype(np.float32) for r in res.results],
        axis=0,
    )
    return np.ascontiguousarray(out)


# revision 22
# speedup vs baseline: 1.3454x; 1.1082x over previous
"""CrossAttention (text+image context, 16 heads) on 8 Trainium2 NeuronCores.

Sharding: data-parallel over batch (16 batches -> 2 per core). No collectives.

Math per batch b (reference):
  q = x @ Wq                      [2048, 1024] -> heads [2048, 16, 64]
  k/v  = ctx_txt @ Wk/Wv          (77 text tokens)
  k2/v2= ctx_img @ Wk_ip/Wv_ip    (256 image tokens)
  out  = softmax(q k^T / 8) v + softmax(q k2^T / 8) v2
  y    = out @ Wo + bo

Device layout choices:
  - All matmul operands bf16 (fp32 psum accumulate). 4x faster PE than fp32.
  - Host pre-transposes x/context so activations arrive as xT [d_model, tok].
  - q is produced transposed (qT [inner, tok]); sim computed keys-on-partitions
    (simT [keys, tok]) so exp output feeds attn@v directly as the stationary
    operand; softmax denominator = extra ones-column appended to v.
  - attn@v output lands queries-on-partitions [128q, 65] so normalization is a
    per-partition scale; the 64-d head outputs are PE-transposed back to
    [inner, tok] for the output projection; bias added via per-partition
    tensor_scalar_add on the transposed output. Host un-transposes.
"""

import numpy as np
import ml_dtypes

import concourse.bass as bass
import concourse.bacc as bacc
import concourse.tile as tile
import concourse.mybir as mybir
from concourse import bass_utils

BF16 = mybir.dt.bfloat16
F32 = mybir.dt.float32
NPBF16 = ml_dtypes.bfloat16

B, N, D = 16, 2048, 1024        # batch, query tokens, model dim (= inner dim)
H, DH = 16, 64                  # heads, head dim
LT, LI, L = 77, 256, 333        # text len, image len, total context len
NCORES = 8
BL = B // NCORES                # 2 batches per core
P = 128
TC = 512                        # query-token chunk (= one PSUM bank of fp32)
NCH = N // TC                   # 4 chunks per batch
NQS = TC // P                   # 4 query sub-tiles of 128 per chunk
SCALE = DH ** -0.5              # 0.125, folded into the exp() activation

TRACE = False
TMPDIR = None
LAST_RESULT = None
_NC = None


def _build_body(tc, io):
    nc = tc.nc
    import contextlib
    ctx = contextlib.ExitStack()
    with ctx:
        # ---------------- persistent pools ----------------
        constp = ctx.enter_context(tc.tile_pool(name="const", bufs=1))
        wq = constp.tile([P, 8, D], BF16, name="wq")
        wo = constp.tile([P, 8, D], BF16, name="wo")
        ident = constp.tile([P, P], BF16, name="ident")
        bot = constp.tile([P, 8], F32, name="bot")
        nc.sync.dma_start(ident[:], io["ident"].ap())
        nc.sync.dma_start(bot[:], io["boT"].ap())

        kvp = ctx.enter_context(tc.tile_pool(name="kvout", bufs=1))
        kt_txt, kt_img, v_txt, v_img0, v_img1 = [], [], [], [], []
        for b in range(BL):
            kt_txt.append(kvp.tile([P, 8, LT], BF16, name=f"kttxt{b}"))
            kt_img.append(kvp.tile([P, 8, LI], BF16, name=f"ktimg{b}"))
            v_txt.append(kvp.tile([P, H * 65], BF16, name=f"vtxt{b}"))
            v_img0.append(kvp.tile([P, H * 65], BF16, name=f"vimg0{b}"))
            v_img1.append(kvp.tile([P, H * 65], BF16, name=f"vimg1{b}"))

        # ---------------- phase 2 pools opened early (xt prefetch) --------
        xqp = ctx.enter_context(tc.tile_pool(name="xq", bufs=2))
        xTrs = [io["xT"].ap()[b].rearrange("(kc p) n -> p kc n", p=P)
                for b in range(BL)]
        xts = {}

        def load_xt(b, ch):
            t = xqp.tile([P, 8, TC], BF16, tag="xt", name=f"xt{b}_{ch}")
            nc.sync.dma_start(t[:], xTrs[b][:, :, ch * TC:(ch + 1) * TC])
            xts[(b, ch)] = t

        # ---------------- phase 1: kv projections ----------------
        # DMA issue order matters (single sync queue): context + kv weights
        # first, then the first x chunk, then Wq; Wo (needed last) at the end.
        with (
            tc.tile_pool(name="kvw", bufs=1) as kvwp,
            tc.tile_pool(name="kvps", bufs=2, space="PSUM") as kvpp,
        ):
            wk = kvwp.tile([P, 8, D], BF16, name="wk")
            wv = kvwp.tile([P, 8, D], BF16, name="wv")
            wkip = kvwp.tile([P, 8, D], BF16, name="wkip")
            wvip = kvwp.tile([P, 8, D], BF16, name="wvip")
            ctxTs = []
            for b in range(BL):
                ctxTs.append(
                    kvwp.tile([P, 8, L], BF16, tag="ctxT", name=f"ctxT{b}", bufs=2)
                )
                nc.sync.dma_start(
                    ctxTs[b][:], io["cT"].ap()[b].rearrange("(kc p) l -> p kc l", p=P)
                )
            for kc in range(8):
                for wt, nm in ((wk, "wk"), (wkip, "wkip")):
                    nc.sync.dma_start(
                        wt[:, kc, :], io[nm].ap()[kc * P:(kc + 1) * P, :]
                    )
            for kc in range(8):
                for wt, nm in ((wv, "wv"), (wvip, "wvip")):
                    nc.sync.dma_start(
                        wt[:, kc, :], io[nm].ap()[kc * P:(kc + 1) * P, :]
                    )
            load_xt(0, 0)
            for kc in range(8):
                nc.sync.dma_start(wq[:, kc, :], io["wq"].ap()[kc * P:(kc + 1) * P, :])
            nc.sync.dma_start(ident[:], io["ident"].ap())
            nc.sync.dma_start(bot[:], io["boT"].ap())
            for kc in range(8):
                nc.sync.dma_start(wo[:, kc, :], io["wo"].ap()[kc * P:(kc + 1) * P, :])

            for b in range(BL):
                ctxT = ctxTs[b]
                # kT projections: out [inner-chunk, keys]
                for ic in range(8):
                    ps = kvpp.tile([P, L], F32, tag="kvk")
                    for kc in range(8):
                        nc.tensor.matmul(
                            ps[:, 0:LT],
                            wk[:, kc, ic * P:(ic + 1) * P],
                            ctxT[:, kc, 0:LT],
                            start=(kc == 0), stop=(kc == 7),
                        )
                    for kc in range(8):
                        nc.tensor.matmul(
                            ps[:, LT:L],
                            wkip[:, kc, ic * P:(ic + 1) * P],
                            ctxT[:, kc, LT:L],
                            start=(kc == 0), stop=(kc == 7),
                        )
                    nc.scalar.copy(kt_txt[b][:, ic, :], ps[:, 0:LT])
                    nc.scalar.copy(kt_img[b][:, ic, :], ps[:, LT:L])
                # v projections: out [keys, inner], evacuated into 65-col head
                # blocks (col 64 of each block later memset to 1.0 -> rowsums)
                for (vout, off, tl, wgt) in (
                    (v_txt[b], 0, LT, wv),
                    (v_img0[b], LT, P, wvip),
                    (v_img1[b], LT + P, P, wvip),
                ):
                    for nh in range(2):
                        ps2 = kvpp.tile([P, TC], F32, tag="kvv")
                        for kc in range(8):
                            nc.tensor.matmul(
                                ps2[0:tl, :],
                                ctxT[:, kc, off:off + tl],
                                wgt[:, kc, nh * TC:(nh + 1) * TC],
                                start=(kc == 0), stop=(kc == 7),
                            )
                        vview = vout[:].rearrange("p (h c) -> p h c", c=65)
                        psview = ps2[:].rearrange("p (h c) -> p h c", c=64)
                        nc.vector.tensor_copy(
                            vview[0:tl, nh * 8:(nh + 1) * 8, 0:64],
                            psview[0:tl, :, :],
                        )
                    vview = vout[:].rearrange("p (h c) -> p h c", c=65)
                    nc.vector.memset(vview[0:tl, :, 64:65], 1.0)

        # ---------------- phase 2: q / attention / out projection ----------------
        # PSUM (8 banks total): "sim" rotates 4 single-bank tiles (sim txt,
        # sim img x2, and the chunk-end transposes), "av" holds txt+img
        # accumulators (2 banks), "fx" rotates q/out-projection accumulators.
        with (
            tc.tile_pool(name="big", bufs=2) as bigp,
            tc.tile_pool(name="sm", bufs=3) as smp,
            tc.tile_pool(name="fxp", bufs=2, space="PSUM") as gpsp,
            tc.tile_pool(name="sps", bufs=4, space="PSUM") as spsp,
            tc.tile_pool(name="avs", bufs=1, space="PSUM") as avsp,
        ):
            oTrs = [io["outT"].ap()[b].rearrange("(oc p) n -> p oc n", p=P)
                    for b in range(BL)]
            pairs = [(b, ch) for b in range(BL) for ch in range(NCH)]
            for pi, (b, chk) in enumerate(pairs):
                    oTr = oTrs[b]
                    tsl = slice(chk * TC, (chk + 1) * TC)
                    if pi + 1 < len(pairs):
                        load_xt(*pairs[pi + 1])
                    xt = xts.pop((b, chk))

                    # q projection, transposed output: qT [inner, tok]
                    qt = bigp.tile([P, 8, TC], BF16, tag="qt")
                    for ic in range(8):
                        ps = gpsp.tile([P, TC], F32, tag="qp", bufs=1)
                        for kc in range(8):
                            nc.tensor.matmul(
                                ps[:],
                                wq[:, kc, ic * P:(ic + 1) * P],
                                xt[:, kc, :],
                                start=(kc == 0), stop=(kc == 7),
                            )
                        nc.scalar.copy(qt[:, ic, :], ps[:])

                    attnA = bigp.tile([P, NQS, H, DH], BF16, tag="attnA")
                    for h in range(H):
                        ic, po = h // 2, (h % 2) * DH
                        qh = qt[po:po + DH, ic, :]            # [64, TC]
                        # simT = k q^T  (keys on partitions)
                        pst = spsp.tile([P, TC], F32, tag="sim")
                        nc.tensor.matmul(
                            pst[0:LT, :], kt_txt[b][po:po + DH, ic, :], qh,
                            start=True, stop=True,
                        )
                        et = smp.tile([P, TC], BF16, tag="et")
                        nc.scalar.activation(
                            et[0:LT, :], pst[0:LT, :],
                            mybir.ActivationFunctionType.Exp, scale=SCALE,
                        )
                        ei = smp.tile([P, 2, TC], BF16, tag="ei")
                        for k2 in range(2):
                            psi = spsp.tile([P, TC], F32, tag="sim")
                            nc.tensor.matmul(
                                psi[:],
                                kt_img[b][po:po + DH, ic, k2 * P:(k2 + 1) * P],
                                qh,
                                start=True, stop=True,
                            )
                            nc.scalar.activation(
                                ei[:, k2, :], psi[:],
                                mybir.ActivationFunctionType.Exp, scale=SCALE,
                            )
                        # attn @ [v | 1]: out [128q, 65] per query sub-tile;
                        # txt group in bank 0, img group in bank 1
                        av = avsp.tile([P, 2, TC], F32, tag="av")
                        for qs in range(NQS):
                            csl = slice(qs * 65, qs * 65 + 65)
                            nc.tensor.matmul(
                                av[:, 0, csl],
                                et[0:LT, qs * P:(qs + 1) * P],
                                v_txt[b][0:LT, h * 65:(h + 1) * 65],
                                start=True, stop=True,
                            )
                            nc.tensor.matmul(
                                av[:, 1, csl],
                                ei[:, 0, qs * P:(qs + 1) * P],
                                v_img0[b][:, h * 65:(h + 1) * 65],
                                start=True, stop=False,
                            )
                            nc.tensor.matmul(
                                av[:, 1, csl],
                                ei[:, 1, qs * P:(qs + 1) * P],
                                v_img1[b][:, h * 65:(h + 1) * 65],
                                start=False, stop=True,
                            )
                        # normalize (per-partition = per-query) and sum txt+img
                        r2 = smp.tile([P, 2, NQS], F32, tag="r2")
                        tt = smp.tile([P, 2, NQS, DH], BF16, tag="tt")
                        for g in range(2):
                            avg = av[:, g, 0:NQS * 65].rearrange(
                                "p (q c) -> p q c", c=65
                            )
                            nc.vector.reciprocal(r2[:, g, :], avg[:, :, 64])
                            nc.vector.tensor_tensor(
                                tt[:, g],
                                avg[:, :, 0:DH],
                                r2[:, g, :].to_broadcast((P, NQS, DH)),
                                op=mybir.AluOpType.mult,
                            )
                        nc.vector.tensor_add(attnA[:, :, h, :], tt[:, 0], tt[:, 1])

                    # transpose attn back to [inner, tok] in 128x128 blocks
                    attnT = bigp.tile([P, 8, TC], BF16, tag="attnT")
                    for qs in range(NQS):
                        for hg in range(2):
                            pt = spsp.tile([P, 4, P], BF16, tag="sim")
                            for j in range(4):
                                hc = hg * 4 + j
                                nc.tensor.transpose(
                                    pt[:, j, :],
                                    attnA[:, qs, 2 * hc:2 * hc + 2, :],
                                    ident[:],
                                )
                            nc.vector.tensor_copy(
                                attnT[:, hg * 4:(hg + 1) * 4, qs * P:(qs + 1) * P],
                                pt[:],
                            )

                    # output projection (transposed): yT = Wo^T attnT + bo
                    osb = bigp.tile([P, 8, TC], BF16, tag="osb")
                    for oc in range(8):
                        ps = gpsp.tile([P, TC], F32, tag="op", bufs=1)
                        for ic in range(8):
                            nc.tensor.matmul(
                                ps[:],
                                wo[:, ic, oc * P:(oc + 1) * P],
                                attnT[:, ic, :],
                                start=(ic == 0), stop=(ic == 7),
                            )
                        nc.vector.tensor_scalar_add(
                            osb[:, oc, :], ps[:], bot[:, oc:oc + 1]
                        )
                    nc.sync.dma_start(oTr[:, :, tsl], osb[:])


def build():
    global _NC
    if _NC is not None:
        return _NC
    nc = bacc.Bacc("TRN2", target_bir_lowering=False, debug=False,
                   num_devices=NCORES)
    io = {
        "xT": nc.dram_tensor("xT", [BL, D, N], BF16, kind="ExternalInput"),
        "cT": nc.dram_tensor("cT", [BL, D, L], BF16, kind="ExternalInput"),
        "wq": nc.dram_tensor("wq", [D, D], BF16, kind="ExternalInput"),
        "wk": nc.dram_tensor("wk", [D, D], BF16, kind="ExternalInput"),
        "wv": nc.dram_tensor("wv", [D, D], BF16, kind="ExternalInput"),
        "wkip": nc.dram_tensor("wkip", [D, D], BF16, kind="ExternalInput"),
        "wvip": nc.dram_tensor("wvip", [D, D], BF16, kind="ExternalInput"),
        "wo": nc.dram_tensor("wo", [D, D], BF16, kind="ExternalInput"),
        "boT": nc.dram_tensor("boT", [P, 8], F32, kind="ExternalInput"),
        "ident": nc.dram_tensor("ident", [P, P], BF16, kind="ExternalInput"),
        "outT": nc.dram_tensor("outT", [BL, D, N], BF16, kind="ExternalOutput"),
    }
    with tile.TileContext(nc) as tc:
        _build_body(tc, io)
    nc.compile()
    _NC = nc
    return nc


def kernel(x, context, Wq, Wk, Wv, Wk_ip, Wv_ip, Wo, bo):
    global LAST_RESULT
    nc = build()

    xT = np.ascontiguousarray(x.astype(NPBF16).transpose(0, 2, 1))
    cT = np.ascontiguousarray(context.astype(NPBF16).transpose(0, 2, 1))
    shared = {
        "wq": np.ascontiguousarray(Wq.astype(NPBF16)),
        "wk": np.ascontiguousarray(Wk.astype(NPBF16)),
        "wv": np.ascontiguousarray(Wv.astype(NPBF16)),
        "wkip": np.ascontiguousarray(Wk_ip.astype(NPBF16)),
        "wvip": np.ascontiguousarray(Wv_ip.astype(NPBF16)),
        "wo": np.ascontiguousarray(Wo.astype(NPBF16)),
        "boT": np.ascontiguousarray(bo.astype(np.float32).reshape(8, P).T),
        "ident": np.eye(P, dtype=NPBF16),
    }
    in_maps = [
        {"xT": xT[c * BL:(c + 1) * BL], "cT": cT[c * BL:(c + 1) * BL], **shared}
        for c in range(NCORES)
    ]
    res = bass_utils.run_bass_kernel_spmd(
        nc, in_maps, core_ids=list(range(NCORES)), trace=TRACE, tmpdir=TMPDIR
    )
    LAST_RESULT = res
    out = np.concatenate(
        [r["outT"].transpose(0, 2, 1).astype(np.float32) for r in res.results],
        axis=0,
    )
    return np.ascontiguousarray(out)


# revision 28
# speedup vs baseline: 1.5123x; 1.1241x over previous
"""CrossAttention (text+image context, 16 heads) on 8 Trainium2 NeuronCores.

Sharding: data-parallel over batch (16 batches -> 2 per core). No collectives.

Math per batch b (reference):
  q = x @ Wq                      [2048, 1024] -> heads [2048, 16, 64]
  k/v  = ctx_txt @ Wk/Wv          (77 text tokens)
  k2/v2= ctx_img @ Wk_ip/Wv_ip    (256 image tokens)
  out  = softmax(q k^T / 8) v + softmax(q k2^T / 8) v2
  y    = out @ Wo + bo

Device layout choices:
  - All matmul operands bf16 (fp32 psum accumulate). 4x faster PE than fp32.
  - Host pre-transposes x/context so activations arrive as xT [d_model, tok].
  - q is produced transposed (qT [inner, tok]); sim computed keys-on-partitions
    (simT [keys, tok]) so exp output feeds attn@v directly as the stationary
    operand; softmax denominator = extra ones-column appended to v.
  - attn@v output lands queries-on-partitions [128q, 65] so normalization is a
    per-partition scale; the 64-d head outputs are PE-transposed back to
    [inner, tok] for the output projection; bias added via per-partition
    tensor_scalar_add on the transposed output. Host un-transposes.
  - Next chunk's q-projection matmuls are interleaved into the attention head
    loop: independent GEMM work fills exp-wait bubbles and keeps the PE HAM
    clock gate warm.
"""

import numpy as np
import ml_dtypes

import concourse.bass as bass
import concourse.bacc as bacc
import concourse.tile as tile
import concourse.mybir as mybir
from concourse import bass_utils

BF16 = mybir.dt.bfloat16
F32 = mybir.dt.float32
NPBF16 = ml_dtypes.bfloat16

B, N, D = 16, 2048, 1024        # batch, query tokens, model dim (= inner dim)
H, DH = 16, 64                  # heads, head dim
LT, LI, L = 77, 256, 333        # text len, image len, total context len
NCORES = 8
BL = B // NCORES                # 2 batches per core
P = 128
TC = 512                        # query-token chunk (= one PSUM bank of fp32)
NCH = N // TC                   # 4 chunks per batch
NQS = TC // P                   # 4 query sub-tiles of 128 per chunk
SCALE = DH ** -0.5              # 0.125, folded into the exp() activation

TRACE = False
TMPDIR = None
LAST_RESULT = None
_NC = None


def _build_body(tc, io):
    nc = tc.nc
    import contextlib
    ctx = contextlib.ExitStack()
    with ctx:
        # ---------------- pools ----------------
        constp = ctx.enter_context(tc.tile_pool(name="const", bufs=1))
        wq = constp.tile([P, 8, D], BF16, name="wq")
        wo = constp.tile([P, 8, D], BF16, name="wo")
        ident = constp.tile([P, P], BF16, name="ident")
        bot = constp.tile([P, 8], F32, name="bot")

        kvp = ctx.enter_context(tc.tile_pool(name="kvout", bufs=1))
        kt_txt, kt_img, v_txt, v_img0, v_img1 = [], [], [], [], []
        for b in range(BL):
            kt_txt.append(kvp.tile([P, 8, LT], BF16, name=f"kttxt{b}"))
            kt_img.append(kvp.tile([P, 8, LI], BF16, name=f"ktimg{b}"))
            v_txt.append(kvp.tile([P, H * 65], BF16, name=f"vtxt{b}"))
            v_img0.append(kvp.tile([P, H * 65], BF16, name=f"vimg0{b}"))
            v_img1.append(kvp.tile([P, H * 65], BF16, name=f"vimg1{b}"))

        xqp = ctx.enter_context(tc.tile_pool(name="xq", bufs=2))
        bigp = ctx.enter_context(tc.tile_pool(name="big", bufs=2))
        smp = ctx.enter_context(tc.tile_pool(name="sm", bufs=3))
        # PSUM (8 banks): "sim" rotates 4 single-bank tiles (sim txt/img x2 and
        # chunk-end transposes; kv-phase accumulators too), "av" = txt+img
        # attn@v accumulators (2 banks), qp/op = q/out projection accumulators.
        gpsp = ctx.enter_context(tc.tile_pool(name="fxp", bufs=2, space="PSUM"))
        spsp = ctx.enter_context(tc.tile_pool(name="sps", bufs=4, space="PSUM"))
        avsp = ctx.enter_context(tc.tile_pool(name="avs", bufs=1, space="PSUM"))

        xTrs = [io["xT"].ap()[b].rearrange("(kc p) n -> p kc n", p=P)
                for b in range(BL)]
        oTrs = [io["outT"].ap()[b].rearrange("(oc p) n -> p oc n", p=P)
                for b in range(BL)]
        pairs = [(b, ch) for b in range(BL) for ch in range(NCH)]
        xts, qts = {}, {}

        def load_xt(b, ch):
            t = xqp.tile([P, 8, TC], BF16, tag="xt", name=f"xt{b}_{ch}")
            nc.sync.dma_start(t[:], xTrs[b][:, :, ch * TC:(ch + 1) * TC])
            xts[(b, ch)] = t

        def qproj_ic(b, ch, ic):
            """Emit one inner-chunk of the q projection for (b, ch)."""
            if (b, ch) not in qts:
                qts[(b, ch)] = bigp.tile(
                    [P, 8, TC], BF16, tag="qt", name=f"qt{b}_{ch}"
                )
            qt = qts[(b, ch)]
            xt = xts[(b, ch)]
            ps = gpsp.tile([P, TC], F32, tag="qp", bufs=1)
            for kc in range(8):
                nc.tensor.matmul(
                    ps[:], wq[:, kc, ic * P:(ic + 1) * P], xt[:, kc, :],
                    start=(kc == 0), stop=(kc == 7),
                )
            nc.scalar.copy(qt[:, ic, :], ps[:])

        # ---------------- DMA issue order (single sync queue) -------------
        # First chunk's x + Wq (feeds the hoisted chunk-0 q projection), then
        # context + kv weights, then Wo (needed last).
        load_xt(0, 0)
        for kc in range(8):
            nc.sync.dma_start(wq[:, kc, :], io["wq"].ap()[kc * P:(kc + 1) * P, :])

        with tc.tile_pool(name="kvw", bufs=1) as kvwp:
            wk = kvwp.tile([P, 8, D], BF16, name="wk")
            wv = kvwp.tile([P, 8, D], BF16, name="wv")
            wkip = kvwp.tile([P, 8, D], BF16, name="wkip")
            wvip = kvwp.tile([P, 8, D], BF16, name="wvip")
            ctxTs = []
            for b in range(BL):
                ctxTs.append(
                    kvwp.tile([P, 8, L], BF16, tag="ctxT", name=f"ctxT{b}", bufs=2)
                )
                nc.sync.dma_start(
                    ctxTs[b][:], io["cT"].ap()[b].rearrange("(kc p) l -> p kc l", p=P)
                )
            for kc in range(8):
                for wt, nm in ((wk, "wk"), (wkip, "wkip")):
                    nc.sync.dma_start(
                        wt[:, kc, :], io[nm].ap()[kc * P:(kc + 1) * P, :]
                    )
            for kc in range(8):
                for wt, nm in ((wv, "wv"), (wvip, "wvip")):
                    nc.sync.dma_start(
                        wt[:, kc, :], io[nm].ap()[kc * P:(kc + 1) * P, :]
                    )
            nc.sync.dma_start(ident[:], io["ident"].ap())
            nc.sync.dma_start(bot[:], io["boT"].ap())
            for kc in range(8):
                nc.sync.dma_start(wo[:, kc, :], io["wo"].ap()[kc * P:(kc + 1) * P, :])
            load_xt(*pairs[1])

            # Hoisted chunk-0 q projection: PE works while kv weights stream.
            for ic in range(8):
                qproj_ic(0, 0, ic)

            # ---------------- phase 1: kv projections ----------------
            for b in range(BL):
                ctxT = ctxTs[b]
                # kT projections: out [inner-chunk, keys]
                for ic in range(8):
                    ps = spsp.tile([P, L], F32, tag="sim", name=f"kvk{b}_{ic}")
                    for kc in range(8):
                        nc.tensor.matmul(
                            ps[:, 0:LT],
                            wk[:, kc, ic * P:(ic + 1) * P],
                            ctxT[:, kc, 0:LT],
                            start=(kc == 0), stop=(kc == 7),
                        )
                    for kc in range(8):
                        nc.tensor.matmul(
                            ps[:, LT:L],
                            wkip[:, kc, ic * P:(ic + 1) * P],
                            ctxT[:, kc, LT:L],
                            start=(kc == 0), stop=(kc == 7),
                        )
                    nc.scalar.copy(kt_txt[b][:, ic, :], ps[:, 0:LT])
                    nc.scalar.copy(kt_img[b][:, ic, :], ps[:, LT:L])
                # v projections: out [keys, inner], evacuated into 65-col head
                # blocks (col 64 of each block later memset to 1.0 -> rowsums)
                for (vout, off, tl, wgt) in (
                    (v_txt[b], 0, LT, wv),
                    (v_img0[b], LT, P, wvip),
                    (v_img1[b], LT + P, P, wvip),
                ):
                    for nh in range(2):
                        ps2 = spsp.tile([P, TC], F32, tag="sim",
                                        name=f"kvv{b}_{off}_{nh}")
                        for kc in range(8):
                            nc.tensor.matmul(
                                ps2[0:tl, :],
                                ctxT[:, kc, off:off + tl],
                                wgt[:, kc, nh * TC:(nh + 1) * TC],
                                start=(kc == 0), stop=(kc == 7),
                            )
                        vview = vout[:].rearrange("p (h c) -> p h c", c=65)
                        psview = ps2[:].rearrange("p (h c) -> p h c", c=64)
                        nc.vector.tensor_copy(
                            vview[0:tl, nh * 8:(nh + 1) * 8, 0:64],
                            psview[0:tl, :, :],
                        )
                    vview = vout[:].rearrange("p (h c) -> p h c", c=65)
                    nc.vector.memset(vview[0:tl, :, 64:65], 1.0)

        # ---------------- phase 2: attention / out projection -------------
        for pi, (b, chk) in enumerate(pairs):
            oTr = oTrs[b]
            tsl = slice(chk * TC, (chk + 1) * TC)
            if pi + 2 < len(pairs):
                load_xt(*pairs[pi + 2])
            xts.pop((b, chk))
            qt = qts.pop((b, chk))
            nxt = pairs[pi + 1] if pi + 1 < len(pairs) else None

            attnA = bigp.tile([P, NQS, H, DH], BF16, tag="attnA")
            for h in range(H):
                # interleave next chunk's q projection between heads:
                # independent GEMM work fills exp-wait bubbles on the PE.
                if nxt is not None and h % 2 == 1:
                    qproj_ic(nxt[0], nxt[1], h // 2)
                ic, po = h // 2, (h % 2) * DH
                qh = qt[po:po + DH, ic, :]            # [64, TC]
                # simT = k q^T  (keys on partitions)
                pst = spsp.tile([P, TC], F32, tag="sim")
                nc.tensor.matmul(
                    pst[0:LT, :], kt_txt[b][po:po + DH, ic, :], qh,
                    start=True, stop=True,
                )
                et = smp.tile([P, TC], BF16, tag="et")
                nc.scalar.activation(
                    et[0:LT, :], pst[0:LT, :],
                    mybir.ActivationFunctionType.Exp, scale=SCALE,
                )
                ei = smp.tile([P, 2, TC], BF16, tag="ei")
                for k2 in range(2):
                    psi = spsp.tile([P, TC], F32, tag="sim")
                    nc.tensor.matmul(
                        psi[:],
                        kt_img[b][po:po + DH, ic, k2 * P:(k2 + 1) * P],
                        qh,
                        start=True, stop=True,
                    )
                    nc.scalar.activation(
                        ei[:, k2, :], psi[:],
                        mybir.ActivationFunctionType.Exp, scale=SCALE,
                    )
                # attn @ [v | 1]: out [128q, 65] per query sub-tile;
                # txt group in bank 0, img group in bank 1
                av = avsp.tile([P, 2, TC], F32, tag="av")
                for qs in range(NQS):
                    csl = slice(qs * 65, qs * 65 + 65)
                    nc.tensor.matmul(
                        av[:, 0, csl],
                        et[0:LT, qs * P:(qs + 1) * P],
                        v_txt[b][0:LT, h * 65:(h + 1) * 65],
                        start=True, stop=True,
                    )
                    nc.tensor.matmul(
                        av[:, 1, csl],
                        ei[:, 0, qs * P:(qs + 1) * P],
                        v_img0[b][:, h * 65:(h + 1) * 65],
                        start=True, stop=False,
                    )
                    nc.tensor.matmul(
                        av[:, 1, csl],
                        ei[:, 1, qs * P:(qs + 1) * P],
                        v_img1[b][:, h * 65:(h + 1) * 65],
                        start=False, stop=True,
                    )
                # normalize (per-partition = per-query) and sum txt+img
                avg = av[:, :, 0:NQS * 65].rearrange(
                    "p g (q c) -> p g q c", c=65
                )
                r2 = smp.tile([P, 2, NQS], F32, tag="r2")
                tt = smp.tile([P, 2, NQS, DH], BF16, tag="tt")
                nc.vector.reciprocal(r2[:], avg[:, :, :, 64])
                nc.vector.tensor_tensor(
                    tt[:],
                    avg[:, :, :, 0:DH],
                    r2[:].to_broadcast((P, 2, NQS, DH)),
                    op=mybir.AluOpType.mult,
                )
                nc.vector.tensor_add(attnA[:, :, h, :], tt[:, 0], tt[:, 1])

            # transpose attn back to [inner, tok] in 128x128 blocks
            attnT = bigp.tile([P, 8, TC], BF16, tag="attnT", bufs=1)
            for qs in range(NQS):
                for hg in range(2):
                    pt = spsp.tile([P, 4, P], BF16, tag="sim")
                    for j in range(4):
                        hc = hg * 4 + j
                        nc.tensor.transpose(
                            pt[:, j, :],
                            attnA[:, qs, 2 * hc:2 * hc + 2, :],
                            ident[:],
                        )
                    nc.vector.tensor_copy(
                        attnT[:, hg * 4:(hg + 1) * 4, qs * P:(qs + 1) * P],
                        pt[:],
                    )

            # output projection (transposed): yT = Wo^T attnT + bo
            osb = bigp.tile([P, 8, TC], BF16, tag="osb", bufs=1)
            for oc in range(8):
                ps = gpsp.tile([P, TC], F32, tag="op", bufs=1)
                for ic in range(8):
                    nc.tensor.matmul(
                        ps[:],
                        wo[:, ic, oc * P:(oc + 1) * P],
                        attnT[:, ic, :],
                        start=(ic == 0), stop=(ic == 7),
                    )
                nc.vector.tensor_scalar_add(
                    osb[:, oc, :], ps[:], bot[:, oc:oc + 1]
                )
            nc.sync.dma_start(oTr[:, :, tsl], osb[:])


def build():
    global _NC
    if _NC is not None:
        return _NC
    nc = bacc.Bacc("TRN2", target_bir_lowering=False, debug=False,
                   num_devices=NCORES)
    io = {
        "xT": nc.dram_tensor("xT", [BL, D, N], BF16, kind="ExternalInput"),
        "cT": nc.dram_tensor("cT", [BL, D, L], BF16, kind="ExternalInput"),
        "wq": nc.dram_tensor("wq", [D, D], BF16, kind="ExternalInput"),
        "wk": nc.dram_tensor("wk", [D, D], BF16, kind="ExternalInput"),
        "wv": nc.dram_tensor("wv", [D, D], BF16, kind="ExternalInput"),
        "wkip": nc.dram_tensor("wkip", [D, D], BF16, kind="ExternalInput"),
        "wvip": nc.dram_tensor("wvip", [D, D], BF16, kind="ExternalInput"),
        "wo": nc.dram_tensor("wo", [D, D], BF16, kind="ExternalInput"),
        "boT": nc.dram_tensor("boT", [P, 8], F32, kind="ExternalInput"),
        "ident": nc.dram_tensor("ident", [P, P], BF16, kind="ExternalInput"),
        "outT": nc.dram_tensor("outT", [BL, D, N], BF16, kind="ExternalOutput"),
    }
    with tile.TileContext(nc) as tc:
        _build_body(tc, io)
    nc.compile()
    _NC = nc
    return nc


def kernel(x, context, Wq, Wk, Wv, Wk_ip, Wv_ip, Wo, bo):
    global LAST_RESULT
    nc = build()

    xT = np.ascontiguousarray(x.astype(NPBF16).transpose(0, 2, 1))
    cT = np.ascontiguousarray(context.astype(NPBF16).transpose(0, 2, 1))
    shared = {
        "wq": np.ascontiguousarray(Wq.astype(NPBF16)),
        "wk": np.ascontiguousarray(Wk.astype(NPBF16)),
        "wv": np.ascontiguousarray(Wv.astype(NPBF16)),
        "wkip": np.ascontiguousarray(Wk_ip.astype(NPBF16)),
        "wvip": np.ascontiguousarray(Wv_ip.astype(NPBF16)),
        "wo": np.ascontiguousarray(Wo.astype(NPBF16)),
        "boT": np.ascontiguousarray(bo.astype(np.float32).reshape(8, P).T),
        "ident": np.eye(P, dtype=NPBF16),
    }
    in_maps = [
        {"xT": xT[c * BL:(c + 1) * BL], "cT": cT[c * BL:(c + 1) * BL], **shared}
        for c in range(NCORES)
    ]
    res = bass_utils.run_bass_kernel_spmd(
        nc, in_maps, core_ids=list(range(NCORES)), trace=TRACE, tmpdir=TMPDIR
    )
    LAST_RESULT = res
    out = np.concatenate(
        [r["outT"].transpose(0, 2, 1).astype(np.float32) for r in res.results],
        axis=0,
    )
    return np.ascontiguousarray(out)


# revision 30
# speedup vs baseline: 1.5599x; 1.0314x over previous
"""CrossAttention (text+image context, 16 heads) on 8 Trainium2 NeuronCores.

Sharding: data-parallel over batch (16 batches -> 2 per core). No collectives.

Math per batch b (reference):
  q = x @ Wq                      [2048, 1024] -> heads [2048, 16, 64]
  k/v  = ctx_txt @ Wk/Wv          (77 text tokens)
  k2/v2= ctx_img @ Wk_ip/Wv_ip    (256 image tokens)
  out  = softmax(q k^T / 8) v + softmax(q k2^T / 8) v2
  y    = out @ Wo + bo

Device layout choices:
  - All matmul operands bf16 (fp32 psum accumulate). 4x faster PE than fp32.
  - Host pre-transposes x/context so activations arrive as xT [d_model, tok].
  - q is produced transposed (qT [inner, tok]); sim computed keys-on-partitions
    (simT [keys, tok]) so exp output feeds attn@v directly as the stationary
    operand; softmax denominator = extra ones-column appended to v.
  - attn@v output lands queries-on-partitions [128q, 65] so normalization is a
    per-partition scale; the 64-d head outputs are PE-transposed back to
    [inner, tok] for the output projection; bias added via per-partition
    tensor_scalar_add on the transposed output. Host un-transposes.
  - Next chunk's q-projection matmuls are interleaved into the attention head
    loop: independent GEMM work fills exp-wait bubbles and keeps the PE HAM
    clock gate warm.
"""

import numpy as np
import ml_dtypes

import concourse.bass as bass
import concourse.bacc as bacc
import concourse.tile as tile
import concourse.mybir as mybir
from concourse import bass_utils

BF16 = mybir.dt.bfloat16
F32 = mybir.dt.float32
NPBF16 = ml_dtypes.bfloat16

B, N, D = 16, 2048, 1024        # batch, query tokens, model dim (= inner dim)
H, DH = 16, 64                  # heads, head dim
LT, LI, L = 77, 256, 333        # text len, image len, total context len
NCORES = 8
BL = B // NCORES                # 2 batches per core
P = 128
TC = 512                        # query-token chunk (= one PSUM bank of fp32)
NCH = N // TC                   # 4 chunks per batch
NQS = TC // P                   # 4 query sub-tiles of 128 per chunk
SCALE = DH ** -0.5              # 0.125, folded into the exp() activation

TRACE = False
TMPDIR = None
LAST_RESULT = None
_NC = None


def _build_body(tc, io):
    nc = tc.nc
    import contextlib
    ctx = contextlib.ExitStack()
    with ctx:
        # ---------------- pools ----------------
        constp = ctx.enter_context(tc.tile_pool(name="const", bufs=1))
        wq = constp.tile([P, 8, D], BF16, name="wq")
        wo = constp.tile([P, 8, D], BF16, name="wo")
        ident = constp.tile([P, P], BF16, name="ident")
        bot = constp.tile([P, 8], F32, name="bot")

        kvp = ctx.enter_context(tc.tile_pool(name="kvout", bufs=1))
        kt_txt, kt_img, v_txt, v_img0, v_img1 = [], [], [], [], []
        for b in range(BL):
            kt_txt.append(kvp.tile([P, 8, LT], BF16, name=f"kttxt{b}"))
            kt_img.append(kvp.tile([P, 8, LI], BF16, name=f"ktimg{b}"))
            v_txt.append(kvp.tile([P, H * 65], BF16, name=f"vtxt{b}"))
            v_img0.append(kvp.tile([P, H * 65], BF16, name=f"vimg0{b}"))
            v_img1.append(kvp.tile([P, H * 65], BF16, name=f"vimg1{b}"))

        xqp = ctx.enter_context(tc.tile_pool(name="xq", bufs=2))
        bigp = ctx.enter_context(tc.tile_pool(name="big", bufs=2))
        smp = ctx.enter_context(tc.tile_pool(name="sm", bufs=3))
        # PSUM (8 banks): "sim" rotates 4 single-bank tiles (sim txt/img x2 and
        # chunk-end transposes; kv-phase accumulators too), "av" = txt+img
        # attn@v accumulators (2 banks), qp/op = q/out projection accumulators.
        gpsp = ctx.enter_context(tc.tile_pool(name="fxp", bufs=2, space="PSUM"))
        spsp = ctx.enter_context(tc.tile_pool(name="sps", bufs=4, space="PSUM"))
        avsp = ctx.enter_context(tc.tile_pool(name="avs", bufs=1, space="PSUM"))

        xTrs = [io["xT"].ap()[b].rearrange("(kc p) n -> p kc n", p=P)
                for b in range(BL)]
        oTrs = [io["outT"].ap()[b].rearrange("(oc p) n -> p oc n", p=P)
                for b in range(BL)]
        pairs = [(b, ch) for b in range(BL) for ch in range(NCH)]
        xts, qts = {}, {}

        def load_xt(b, ch):
            t = xqp.tile([P, 8, TC], BF16, tag="xt", name=f"xt{b}_{ch}")
            nc.sync.dma_start(t[:], xTrs[b][:, :, ch * TC:(ch + 1) * TC])
            xts[(b, ch)] = t

        def qproj_ic(b, ch, ic):
            """Emit one inner-chunk of the q projection for (b, ch)."""
            if (b, ch) not in qts:
                qts[(b, ch)] = bigp.tile(
                    [P, 8, TC], BF16, tag="qt", name=f"qt{b}_{ch}"
                )
            qt = qts[(b, ch)]
            xt = xts[(b, ch)]
            ps = gpsp.tile([P, TC], F32, tag="qp", bufs=1)
            for kc in range(8):
                nc.tensor.matmul(
                    ps[:], wq[:, kc, ic * P:(ic + 1) * P], xt[:, kc, :],
                    start=(kc == 0), stop=(kc == 7),
                )
            nc.scalar.copy(qt[:, ic, :], ps[:])

        # ---------------- DMA issue order (single sync queue) -------------
        # First chunk's x + Wq (feeds the hoisted chunk-0 q projection), then
        # context + kv weights, then Wo (needed last).
        load_xt(0, 0)
        for kc in range(8):
            nc.sync.dma_start(wq[:, kc, :], io["wq"].ap()[kc * P:(kc + 1) * P, :])

        with tc.tile_pool(name="kvw", bufs=1) as kvwp:
            wk = kvwp.tile([P, 8, D], BF16, name="wk")
            wv = kvwp.tile([P, 8, D], BF16, name="wv")
            wkip = kvwp.tile([P, 8, D], BF16, name="wkip")
            wvip = kvwp.tile([P, 8, D], BF16, name="wvip")
            ctxTs = []
            for b in range(BL):
                ctxTs.append(
                    kvwp.tile([P, 8, L], BF16, tag="ctxT", name=f"ctxT{b}", bufs=2)
                )
                nc.sync.dma_start(
                    ctxTs[b][:], io["cT"].ap()[b].rearrange("(kc p) l -> p kc l", p=P)
                )
            for kc in range(8):
                for wt, nm in ((wk, "wk"), (wkip, "wkip")):
                    nc.sync.dma_start(
                        wt[:, kc, :], io[nm].ap()[kc * P:(kc + 1) * P, :]
                    )
            for kc in range(8):
                for wt, nm in ((wv, "wv"), (wvip, "wvip")):
                    nc.sync.dma_start(
                        wt[:, kc, :], io[nm].ap()[kc * P:(kc + 1) * P, :]
                    )
            nc.sync.dma_start(ident[:], io["ident"].ap())
            nc.sync.dma_start(bot[:], io["boT"].ap())
            for kc in range(8):
                nc.sync.dma_start(wo[:, kc, :], io["wo"].ap()[kc * P:(kc + 1) * P, :])
            load_xt(*pairs[1])

            # Hoisted chunk-0 q projection: PE works while kv weights stream.
            for ic in range(8):
                qproj_ic(0, 0, ic)

            # ---------------- phase 1: kv projections ----------------
            for b in range(BL):
                ctxT = ctxTs[b]
                # kT projections: out [inner-chunk, keys]
                for ic in range(8):
                    ps = spsp.tile([P, L], F32, tag="sim", name=f"kvk{b}_{ic}")
                    for kc in range(8):
                        nc.tensor.matmul(
                            ps[:, 0:LT],
                            wk[:, kc, ic * P:(ic + 1) * P],
                            ctxT[:, kc, 0:LT],
                            start=(kc == 0), stop=(kc == 7),
                        )
                    for kc in range(8):
                        nc.tensor.matmul(
                            ps[:, LT:L],
                            wkip[:, kc, ic * P:(ic + 1) * P],
                            ctxT[:, kc, LT:L],
                            start=(kc == 0), stop=(kc == 7),
                        )
                    nc.scalar.copy(kt_txt[b][:, ic, :], ps[:, 0:LT])
                    nc.scalar.copy(kt_img[b][:, ic, :], ps[:, LT:L])
                # v projections: out [keys, inner], evacuated into 65-col head
                # blocks (col 64 of each block later memset to 1.0 -> rowsums)
                for (vout, off, tl, wgt) in (
                    (v_txt[b], 0, LT, wv),
                    (v_img0[b], LT, P, wvip),
                    (v_img1[b], LT + P, P, wvip),
                ):
                    for nh in range(2):
                        ps2 = spsp.tile([P, TC], F32, tag="sim",
                                        name=f"kvv{b}_{off}_{nh}")
                        for kc in range(8):
                            nc.tensor.matmul(
                                ps2[0:tl, :],
                                ctxT[:, kc, off:off + tl],
                                wgt[:, kc, nh * TC:(nh + 1) * TC],
                                start=(kc == 0), stop=(kc == 7),
                            )
                        vview = vout[:].rearrange("p (h c) -> p h c", c=65)
                        psview = ps2[:].rearrange("p (h c) -> p h c", c=64)
                        nc.vector.tensor_copy(
                            vview[0:tl, nh * 8:(nh + 1) * 8, 0:64],
                            psview[0:tl, :, :],
                        )
                    vview = vout[:].rearrange("p (h c) -> p h c", c=65)
                    nc.vector.memset(vview[0:tl, :, 64:65], 1.0)

        # ---------------- phase 2: attention / out projection -------------
        attnTs, osbs = {}, {}

        def oproj_oc(b, ch, oc):
            """Emit one out-projection column-chunk for (b, ch)."""
            if (b, ch) not in osbs:
                osbs[(b, ch)] = bigp.tile(
                    [P, 8, TC], BF16, tag="osb", bufs=1, name=f"osb{b}_{ch}"
                )
            osb = osbs[(b, ch)]
            attnT = attnTs[(b, ch)]
            ps = gpsp.tile([P, TC], F32, tag="op", bufs=1)
            for ic in range(8):
                nc.tensor.matmul(
                    ps[:], wo[:, ic, oc * P:(oc + 1) * P], attnT[:, ic, :],
                    start=(ic == 0), stop=(ic == 7),
                )
            nc.vector.tensor_scalar_add(osb[:, oc, :], ps[:], bot[:, oc:oc + 1])

        def store_out(b, ch):
            attnTs.pop((b, ch))
            osb = osbs.pop((b, ch))
            nc.sync.dma_start(
                oTrs[b][:, :, ch * TC:(ch + 1) * TC], osb[:]
            )

        for pi, (b, chk) in enumerate(pairs):
            if pi + 2 < len(pairs):
                load_xt(*pairs[pi + 2])
            xts.pop((b, chk))
            qt = qts.pop((b, chk))
            nxt = pairs[pi + 1] if pi + 1 < len(pairs) else None
            prv = pairs[pi - 1] if pi > 0 else None

            attnA = bigp.tile([P, NQS, H, DH], BF16, tag="attnA")
            for h in range(H):
                # Interleave independent GEMM work between heads to fill
                # exp-wait bubbles on the PE and hide attn@v LDWEIGHTS:
                # odd heads -> next chunk's q projection, even heads ->
                # previous chunk's out projection.
                if nxt is not None and h % 2 == 1:
                    qproj_ic(nxt[0], nxt[1], h // 2)
                if prv is not None and h % 2 == 0:
                    oproj_oc(prv[0], prv[1], h // 2)
                ic, po = h // 2, (h % 2) * DH
                qh = qt[po:po + DH, ic, :]            # [64, TC]
                # simT = k q^T  (keys on partitions)
                pst = spsp.tile([P, TC], F32, tag="sim")
                nc.tensor.matmul(
                    pst[0:LT, :], kt_txt[b][po:po + DH, ic, :], qh,
                    start=True, stop=True,
                )
                et = smp.tile([P, TC], BF16, tag="et")
                nc.scalar.activation(
                    et[0:LT, :], pst[0:LT, :],
                    mybir.ActivationFunctionType.Exp, scale=SCALE,
                )
                ei = smp.tile([P, 2, TC], BF16, tag="ei")
                for k2 in range(2):
                    psi = spsp.tile([P, TC], F32, tag="sim")
                    nc.tensor.matmul(
                        psi[:],
                        kt_img[b][po:po + DH, ic, k2 * P:(k2 + 1) * P],
                        qh,
                        start=True, stop=True,
                    )
                    nc.scalar.activation(
                        ei[:, k2, :], psi[:],
                        mybir.ActivationFunctionType.Exp, scale=SCALE,
                    )
                # attn @ [v | 1]: out [128q, 65] per query sub-tile;
                # txt group in bank 0, img group in bank 1
                av = avsp.tile([P, 2, TC], F32, tag="av")
                for qs in range(NQS):
                    csl = slice(qs * 65, qs * 65 + 65)
                    nc.tensor.matmul(
                        av[:, 0, csl],
                        et[0:LT, qs * P:(qs + 1) * P],
                        v_txt[b][0:LT, h * 65:(h + 1) * 65],
                        start=True, stop=True,
                    )
                    nc.tensor.matmul(
                        av[:, 1, csl],
                        ei[:, 0, qs * P:(qs + 1) * P],
                        v_img0[b][:, h * 65:(h + 1) * 65],
                        start=True, stop=False,
                    )
                    nc.tensor.matmul(
                        av[:, 1, csl],
                        ei[:, 1, qs * P:(qs + 1) * P],
                        v_img1[b][:, h * 65:(h + 1) * 65],
                        start=False, stop=True,
                    )
                # normalize (per-partition = per-query) and sum txt+img
                avg = av[:, :, 0:NQS * 65].rearrange(
                    "p g (q c) -> p g q c", c=65
                )
                r2 = smp.tile([P, 2, NQS], F32, tag="r2")
                tt = smp.tile([P, 2, NQS, DH], BF16, tag="tt")
                nc.vector.reciprocal(r2[:], avg[:, :, :, 64])
                nc.vector.tensor_tensor(
                    tt[:],
                    avg[:, :, :, 0:DH],
                    r2[:].to_broadcast((P, 2, NQS, DH)),
                    op=mybir.AluOpType.mult,
                )
                nc.vector.tensor_add(attnA[:, :, h, :], tt[:, 0], tt[:, 1])

            # previous chunk fully projected by now: store it
            if prv is not None:
                store_out(*prv)

            # transpose attn back to [inner, tok] in 128x128 blocks
            attnT = bigp.tile([P, 8, TC], BF16, tag="attnT", bufs=1,
                              name=f"attnT{b}_{chk}")
            attnTs[(b, chk)] = attnT
            for qs in range(NQS):
                for hg in range(2):
                    pt = spsp.tile([P, 4, P], BF16, tag="sim")
                    for j in range(4):
                        hc = hg * 4 + j
                        nc.tensor.transpose(
                            pt[:, j, :],
                            attnA[:, qs, 2 * hc:2 * hc + 2, :],
                            ident[:],
                        )
                    nc.vector.tensor_copy(
                        attnT[:, hg * 4:(hg + 1) * 4, qs * P:(qs + 1) * P],
                        pt[:],
                    )

        # tail: out projection + store of the final chunk
        last = pairs[-1]
        for oc in range(8):
            oproj_oc(last[0], last[1], oc)
        store_out(*last)


def build():
    global _NC
    if _NC is not None:
        return _NC
    nc = bacc.Bacc("TRN2", target_bir_lowering=False, debug=False,
                   num_devices=NCORES)
    io = {
        "xT": nc.dram_tensor("xT", [BL, D, N], BF16, kind="ExternalInput"),
        "cT": nc.dram_tensor("cT", [BL, D, L], BF16, kind="ExternalInput"),
        "wq": nc.dram_tensor("wq", [D, D], BF16, kind="ExternalInput"),
        "wk": nc.dram_tensor("wk", [D, D], BF16, kind="ExternalInput"),
        "wv": nc.dram_tensor("wv", [D, D], BF16, kind="ExternalInput"),
        "wkip": nc.dram_tensor("wkip", [D, D], BF16, kind="ExternalInput"),
        "wvip": nc.dram_tensor("wvip", [D, D], BF16, kind="ExternalInput"),
        "wo": nc.dram_tensor("wo", [D, D], BF16, kind="ExternalInput"),
        "boT": nc.dram_tensor("boT", [P, 8], F32, kind="ExternalInput"),
        "ident": nc.dram_tensor("ident", [P, P], BF16, kind="ExternalInput"),
        "outT": nc.dram_tensor("outT", [BL, D, N], BF16, kind="ExternalOutput"),
    }
    with tile.TileContext(nc) as tc:
        _build_body(tc, io)
    nc.compile()
    _NC = nc
    return nc


def kernel(x, context, Wq, Wk, Wv, Wk_ip, Wv_ip, Wo, bo):
    global LAST_RESULT
    nc = build()

    xT = np.ascontiguousarray(x.astype(NPBF16).transpose(0, 2, 1))
    cT = np.ascontiguousarray(context.astype(NPBF16).transpose(0, 2, 1))
    shared = {
        "wq": np.ascontiguousarray(Wq.astype(NPBF16)),
        "wk": np.ascontiguousarray(Wk.astype(NPBF16)),
        "wv": np.ascontiguousarray(Wv.astype(NPBF16)),
        "wkip": np.ascontiguousarray(Wk_ip.astype(NPBF16)),
        "wvip": np.ascontiguousarray(Wv_ip.astype(NPBF16)),
        "wo": np.ascontiguousarray(Wo.astype(NPBF16)),
        "boT": np.ascontiguousarray(bo.astype(np.float32).reshape(8, P).T),
        "ident": np.eye(P, dtype=NPBF16),
    }
    in_maps = [
        {"xT": xT[c * BL:(c + 1) * BL], "cT": cT[c * BL:(c + 1) * BL], **shared}
        for c in range(NCORES)
    ]
    res = bass_utils.run_bass_kernel_spmd(
        nc, in_maps, core_ids=list(range(NCORES)), trace=TRACE, tmpdir=TMPDIR
    )
    LAST_RESULT = res
    out = np.concatenate(
        [r["outT"].transpose(0, 2, 1).astype(np.float32) for r in res.results],
        axis=0,
    )
    return np.ascontiguousarray(out)


# revision 32
# speedup vs baseline: 1.5810x; 1.0136x over previous
"""CrossAttention (text+image context, 16 heads) on 8 Trainium2 NeuronCores.

Sharding: data-parallel over batch (16 batches -> 2 per core). No collectives.

Math per batch b (reference):
  q = x @ Wq                      [2048, 1024] -> heads [2048, 16, 64]
  k/v  = ctx_txt @ Wk/Wv          (77 text tokens)
  k2/v2= ctx_img @ Wk_ip/Wv_ip    (256 image tokens)
  out  = softmax(q k^T / 8) v + softmax(q k2^T / 8) v2
  y    = out @ Wo + bo

Device layout choices:
  - All matmul operands bf16 (fp32 psum accumulate). 4x faster PE than fp32.
  - Host pre-transposes x/context so activations arrive as xT [d_model, tok].
  - q is produced transposed (qT [inner, tok]); sim computed keys-on-partitions
    (simT [keys, tok]) so exp output feeds attn@v directly as the stationary
    operand; softmax denominator = extra ones-column appended to v.
  - attn@v output lands queries-on-partitions [128q, 65] so normalization is a
    per-partition scale; the 64-d head outputs are PE-transposed back to
    [inner, tok] for the output projection; bias added via per-partition
    tensor_scalar_add on the transposed output. Host un-transposes.
  - Next chunk's q-projection matmuls are interleaved into the attention head
    loop: independent GEMM work fills exp-wait bubbles and keeps the PE HAM
    clock gate warm.
"""

import numpy as np
import ml_dtypes

import concourse.bass as bass
import concourse.bacc as bacc
import concourse.tile as tile
import concourse.mybir as mybir
from concourse import bass_utils

BF16 = mybir.dt.bfloat16
F32 = mybir.dt.float32
NPBF16 = ml_dtypes.bfloat16

B, N, D = 16, 2048, 1024        # batch, query tokens, model dim (= inner dim)
H, DH = 16, 64                  # heads, head dim
LT, LI, L = 77, 256, 333        # text len, image len, total context len
NCORES = 8
BL = B // NCORES                # 2 batches per core
P = 128
TC = 512                        # query-token chunk (= one PSUM bank of fp32)
NCH = N // TC                   # 4 chunks per batch
NQS = TC // P                   # 4 query sub-tiles of 128 per chunk
SCALE = DH ** -0.5              # 0.125, folded into the exp() activation

TRACE = False
TMPDIR = None
LAST_RESULT = None
_NC = None


def _build_body(tc, io):
    nc = tc.nc
    import contextlib
    ctx = contextlib.ExitStack()
    with ctx:
        # ---------------- pools ----------------
        constp = ctx.enter_context(tc.tile_pool(name="const", bufs=1))
        wq = constp.tile([P, 8, D], BF16, name="wq")
        wo = constp.tile([P, 8, D], BF16, name="wo")
        ident = constp.tile([P, P], BF16, name="ident")
        bot = constp.tile([P, 8], F32, name="bot")

        kvp = ctx.enter_context(tc.tile_pool(name="kvout", bufs=1))
        kt_txt, kt_img, v_txt, v_img0, v_img1 = [], [], [], [], []
        for b in range(BL):
            kt_txt.append(kvp.tile([P, 8, LT], BF16, name=f"kttxt{b}"))
            kt_img.append(kvp.tile([P, 8, LI], BF16, name=f"ktimg{b}"))
            v_txt.append(kvp.tile([P, H * 65], BF16, name=f"vtxt{b}"))
            v_img0.append(kvp.tile([P, H * 65], BF16, name=f"vimg0{b}"))
            v_img1.append(kvp.tile([P, H * 65], BF16, name=f"vimg1{b}"))

        xqp = ctx.enter_context(tc.tile_pool(name="xq", bufs=2))
        bigp = ctx.enter_context(tc.tile_pool(name="big", bufs=2))
        smp = ctx.enter_context(tc.tile_pool(name="sm", bufs=3))
        # PSUM (8 banks): "sim" rotates 4 single-bank tiles (sim txt/img x2 and
        # chunk-end transposes; kv-phase accumulators too), "av" = txt+img
        # attn@v accumulators (2 banks), qp/op = q/out projection accumulators.
        gpsp = ctx.enter_context(tc.tile_pool(name="fxp", bufs=2, space="PSUM"))
        spsp = ctx.enter_context(tc.tile_pool(name="sps", bufs=4, space="PSUM"))
        avsp = ctx.enter_context(tc.tile_pool(name="avs", bufs=1, space="PSUM"))

        xTrs = [io["xT"].ap()[b].rearrange("(kc p) n -> p kc n", p=P)
                for b in range(BL)]
        oTrs = [io["outT"].ap()[b].rearrange("(oc p) n -> p oc n", p=P)
                for b in range(BL)]
        pairs = [(b, ch) for b in range(BL) for ch in range(NCH)]
        xts, qts = {}, {}

        def load_xt(b, ch):
            t = xqp.tile([P, 8, TC], BF16, tag="xt", name=f"xt{b}_{ch}")
            nc.sync.dma_start(t[:], xTrs[b][:, :, ch * TC:(ch + 1) * TC])
            xts[(b, ch)] = t

        def qproj_ic(b, ch, ic):
            """Emit one inner-chunk of the q projection for (b, ch)."""
            if (b, ch) not in qts:
                qts[(b, ch)] = bigp.tile(
                    [P, 8, TC], BF16, tag="qt", name=f"qt{b}_{ch}"
                )
            qt = qts[(b, ch)]
            xt = xts[(b, ch)]
            ps = gpsp.tile([P, TC], F32, tag="qp", bufs=1)
            for kc in range(8):
                nc.tensor.matmul(
                    ps[:], wq[:, kc, ic * P:(ic + 1) * P], xt[:, kc, :],
                    start=(kc == 0), stop=(kc == 7),
                )
            nc.vector.tensor_copy(qt[:, ic, :], ps[:])

        # ---------------- DMA issue order (single sync queue) -------------
        # First chunk's x + Wq (feeds the hoisted chunk-0 q projection), then
        # context + kv weights, then Wo (needed last).
        load_xt(0, 0)
        for kc in range(8):
            nc.sync.dma_start(wq[:, kc, :], io["wq"].ap()[kc * P:(kc + 1) * P, :])

        with tc.tile_pool(name="kvw", bufs=1) as kvwp:
            wk = kvwp.tile([P, 8, D], BF16, name="wk")
            wv = kvwp.tile([P, 8, D], BF16, name="wv")
            wkip = kvwp.tile([P, 8, D], BF16, name="wkip")
            wvip = kvwp.tile([P, 8, D], BF16, name="wvip")
            ctxTs = []
            for b in range(BL):
                ctxTs.append(
                    kvwp.tile([P, 8, L], BF16, tag="ctxT", name=f"ctxT{b}", bufs=2)
                )
                nc.sync.dma_start(
                    ctxTs[b][:], io["cT"].ap()[b].rearrange("(kc p) l -> p kc l", p=P)
                )
            for kc in range(8):
                for wt, nm in ((wk, "wk"), (wkip, "wkip")):
                    nc.sync.dma_start(
                        wt[:, kc, :], io[nm].ap()[kc * P:(kc + 1) * P, :]
                    )
            for kc in range(8):
                for wt, nm in ((wv, "wv"), (wvip, "wvip")):
                    nc.sync.dma_start(
                        wt[:, kc, :], io[nm].ap()[kc * P:(kc + 1) * P, :]
                    )
            nc.sync.dma_start(ident[:], io["ident"].ap())
            nc.sync.dma_start(bot[:], io["boT"].ap())
            for kc in range(8):
                nc.sync.dma_start(wo[:, kc, :], io["wo"].ap()[kc * P:(kc + 1) * P, :])
            load_xt(*pairs[1])

            # Hoisted chunk-0 q projection: PE works while kv weights stream.
            for ic in range(8):
                qproj_ic(0, 0, ic)

            # ---------------- phase 1: kv projections ----------------
            for b in range(BL):
                ctxT = ctxTs[b]
                # kT projections: out [inner-chunk, keys]
                for ic in range(8):
                    ps = spsp.tile([P, L], F32, tag="sim", name=f"kvk{b}_{ic}")
                    for kc in range(8):
                        nc.tensor.matmul(
                            ps[:, 0:LT],
                            wk[:, kc, ic * P:(ic + 1) * P],
                            ctxT[:, kc, 0:LT],
                            start=(kc == 0), stop=(kc == 7),
                        )
                    for kc in range(8):
                        nc.tensor.matmul(
                            ps[:, LT:L],
                            wkip[:, kc, ic * P:(ic + 1) * P],
                            ctxT[:, kc, LT:L],
                            start=(kc == 0), stop=(kc == 7),
                        )
                    nc.scalar.copy(kt_txt[b][:, ic, :], ps[:, 0:LT])
                    nc.scalar.copy(kt_img[b][:, ic, :], ps[:, LT:L])
                # v projections: out [keys, inner], evacuated into 65-col head
                # blocks (col 64 of each block later memset to 1.0 -> rowsums)
                for (vout, off, tl, wgt) in (
                    (v_txt[b], 0, LT, wv),
                    (v_img0[b], LT, P, wvip),
                    (v_img1[b], LT + P, P, wvip),
                ):
                    for nh in range(2):
                        ps2 = spsp.tile([P, TC], F32, tag="sim",
                                        name=f"kvv{b}_{off}_{nh}")
                        for kc in range(8):
                            nc.tensor.matmul(
                                ps2[0:tl, :],
                                ctxT[:, kc, off:off + tl],
                                wgt[:, kc, nh * TC:(nh + 1) * TC],
                                start=(kc == 0), stop=(kc == 7),
                            )
                        vview = vout[:].rearrange("p (h c) -> p h c", c=65)
                        psview = ps2[:].rearrange("p (h c) -> p h c", c=64)
                        nc.vector.tensor_copy(
                            vview[0:tl, nh * 8:(nh + 1) * 8, 0:64],
                            psview[0:tl, :, :],
                        )
                    vview = vout[:].rearrange("p (h c) -> p h c", c=65)
                    nc.vector.memset(vview[0:tl, :, 64:65], 1.0)

        # ---------------- phase 2: attention / out projection -------------
        attnTs, osbs = {}, {}

        def oproj_oc(b, ch, oc):
            """Emit one out-projection column-chunk for (b, ch)."""
            if (b, ch) not in osbs:
                osbs[(b, ch)] = bigp.tile(
                    [P, 8, TC], BF16, tag="osb", bufs=1, name=f"osb{b}_{ch}"
                )
            osb = osbs[(b, ch)]
            attnT = attnTs[(b, ch)]
            ps = gpsp.tile([P, TC], F32, tag="op", bufs=1)
            for ic in range(8):
                nc.tensor.matmul(
                    ps[:], wo[:, ic, oc * P:(oc + 1) * P], attnT[:, ic, :],
                    start=(ic == 0), stop=(ic == 7),
                )
            nc.vector.tensor_scalar_add(osb[:, oc, :], ps[:], bot[:, oc:oc + 1])

        def store_out(b, ch):
            attnTs.pop((b, ch))
            osb = osbs.pop((b, ch))
            nc.sync.dma_start(
                oTrs[b][:, :, ch * TC:(ch + 1) * TC], osb[:]
            )

        for pi, (b, chk) in enumerate(pairs):
            if pi + 2 < len(pairs):
                load_xt(*pairs[pi + 2])
            xts.pop((b, chk))
            qt = qts.pop((b, chk))
            nxt = pairs[pi + 1] if pi + 1 < len(pairs) else None
            prv = pairs[pi - 1] if pi > 0 else None

            attnA = bigp.tile([P, NQS, H, DH], BF16, tag="attnA")
            for h in range(H):
                # Interleave independent GEMM work between heads to fill
                # exp-wait bubbles on the PE and hide attn@v LDWEIGHTS:
                # odd heads -> next chunk's q projection, even heads ->
                # previous chunk's out projection.
                if nxt is not None and h % 2 == 1:
                    qproj_ic(nxt[0], nxt[1], h // 2)
                if prv is not None and h % 2 == 0:
                    oproj_oc(prv[0], prv[1], h // 2)
                ic, po = h // 2, (h % 2) * DH
                qh = qt[po:po + DH, ic, :]            # [64, TC]
                # simT = k q^T  (keys on partitions)
                pst = spsp.tile([P, TC], F32, tag="sim")
                nc.tensor.matmul(
                    pst[0:LT, :], kt_txt[b][po:po + DH, ic, :], qh,
                    start=True, stop=True,
                )
                et = smp.tile([P, TC], BF16, tag="et")
                nc.scalar.activation(
                    et[0:LT, :], pst[0:LT, :],
                    mybir.ActivationFunctionType.Exp, scale=SCALE,
                )
                ei = smp.tile([P, 2, TC], BF16, tag="ei")
                for k2 in range(2):
                    psi = spsp.tile([P, TC], F32, tag="sim")
                    nc.tensor.matmul(
                        psi[:],
                        kt_img[b][po:po + DH, ic, k2 * P:(k2 + 1) * P],
                        qh,
                        start=True, stop=True,
                    )
                    nc.scalar.activation(
                        ei[:, k2, :], psi[:],
                        mybir.ActivationFunctionType.Exp, scale=SCALE,
                    )
                # attn @ [v | 1]: out [128q, 65] per query sub-tile;
                # txt group in bank 0, img group in bank 1
                av = avsp.tile([P, 2, TC], F32, tag="av")
                for qs in range(NQS):
                    csl = slice(qs * 65, qs * 65 + 65)
                    nc.tensor.matmul(
                        av[:, 0, csl],
                        et[0:LT, qs * P:(qs + 1) * P],
                        v_txt[b][0:LT, h * 65:(h + 1) * 65],
                        start=True, stop=True,
                    )
                    nc.tensor.matmul(
                        av[:, 1, csl],
                        ei[:, 0, qs * P:(qs + 1) * P],
                        v_img0[b][:, h * 65:(h + 1) * 65],
                        start=True, stop=False,
                    )
                    nc.tensor.matmul(
                        av[:, 1, csl],
                        ei[:, 1, qs * P:(qs + 1) * P],
                        v_img1[b][:, h * 65:(h + 1) * 65],
                        start=False, stop=True,
                    )
                # normalize (per-partition = per-query) and sum txt+img
                avg = av[:, :, 0:NQS * 65].rearrange(
                    "p g (q c) -> p g q c", c=65
                )
                r2 = smp.tile([P, 2, NQS], F32, tag="r2")
                tt = smp.tile([P, 2, NQS, DH], BF16, tag="tt")
                nc.vector.reciprocal(r2[:], avg[:, :, :, 64])
                nc.vector.tensor_tensor(
                    tt[:],
                    avg[:, :, :, 0:DH],
                    r2[:].to_broadcast((P, 2, NQS, DH)),
                    op=mybir.AluOpType.mult,
                )
                nc.vector.tensor_add(attnA[:, :, h, :], tt[:, 0], tt[:, 1])

            # previous chunk fully projected by now: store it
            if prv is not None:
                store_out(*prv)

            # transpose attn back to [inner, tok] in 128x128 blocks
            attnT = bigp.tile([P, 8, TC], BF16, tag="attnT", bufs=1,
                              name=f"attnT{b}_{chk}")
            attnTs[(b, chk)] = attnT
            for qs in range(NQS):
                for hg in range(2):
                    pt = spsp.tile([P, 4, P], BF16, tag="sim")
                    for j in range(4):
                        hc = hg * 4 + j
                        nc.tensor.transpose(
                            pt[:, j, :],
                            attnA[:, qs, 2 * hc:2 * hc + 2, :],
                            ident[:],
                        )
                    nc.vector.tensor_copy(
                        attnT[:, hg * 4:(hg + 1) * 4, qs * P:(qs + 1) * P],
                        pt[:],
                    )

        # tail: out projection + store of the final chunk
        last = pairs[-1]
        for oc in range(8):
            oproj_oc(last[0], last[1], oc)
        store_out(*last)


def build():
    global _NC
    if _NC is not None:
        return _NC
    nc = bacc.Bacc("TRN2", target_bir_lowering=False, debug=False,
                   num_devices=NCORES)
    io = {
        "xT": nc.dram_tensor("xT", [BL, D, N], BF16, kind="ExternalInput"),
        "cT": nc.dram_tensor("cT", [BL, D, L], BF16, kind="ExternalInput"),
        "wq": nc.dram_tensor("wq", [D, D], BF16, kind="ExternalInput"),
        "wk": nc.dram_tensor("wk", [D, D], BF16, kind="ExternalInput"),
        "wv": nc.dram_tensor("wv", [D, D], BF16, kind="ExternalInput"),
        "wkip": nc.dram_tensor("wkip", [D, D], BF16, kind="ExternalInput"),
        "wvip": nc.dram_tensor("wvip", [D, D], BF16, kind="ExternalInput"),
        "wo": nc.dram_tensor("wo", [D, D], BF16, kind="ExternalInput"),
        "boT": nc.dram_tensor("boT", [P, 8], F32, kind="ExternalInput"),
        "ident": nc.dram_tensor("ident", [P, P], BF16, kind="ExternalInput"),
        "outT": nc.dram_tensor("outT", [BL, D, N], BF16, kind="ExternalOutput"),
    }
    with tile.TileContext(nc) as tc:
        _build_body(tc, io)
    nc.compile()
    _NC = nc
    return nc


def kernel(x, context, Wq, Wk, Wv, Wk_ip, Wv_ip, Wo, bo):
    global LAST_RESULT
    nc = build()

    xT = np.ascontiguousarray(x.astype(NPBF16).transpose(0, 2, 1))
    cT = np.ascontiguousarray(context.astype(NPBF16).transpose(0, 2, 1))
    shared = {
        "wq": np.ascontiguousarray(Wq.astype(NPBF16)),
        "wk": np.ascontiguousarray(Wk.astype(NPBF16)),
        "wv": np.ascontiguousarray(Wv.astype(NPBF16)),
        "wkip": np.ascontiguousarray(Wk_ip.astype(NPBF16)),
        "wvip": np.ascontiguousarray(Wv_ip.astype(NPBF16)),
        "wo": np.ascontiguousarray(Wo.astype(NPBF16)),
        "boT": np.ascontiguousarray(bo.astype(np.float32).reshape(8, P).T),
        "ident": np.eye(P, dtype=NPBF16),
    }
    in_maps = [
        {"xT": xT[c * BL:(c + 1) * BL], "cT": cT[c * BL:(c + 1) * BL], **shared}
        for c in range(NCORES)
    ]
    res = bass_utils.run_bass_kernel_spmd(
        nc, in_maps, core_ids=list(range(NCORES)), trace=TRACE, tmpdir=TMPDIR
    )
    LAST_RESULT = res
    out = np.concatenate(
        [r["outT"].transpose(0, 2, 1).astype(np.float32) for r in res.results],
        axis=0,
    )
    return np.ascontiguousarray(out)


# revision 33
# speedup vs baseline: 1.5814x; 1.0003x over previous
"""CrossAttention (text+image context, 16 heads) on 8 Trainium2 NeuronCores.

Sharding: data-parallel over batch (16 batches -> 2 per core). No collectives.

Math per batch b (reference):
  q = x @ Wq                      [2048, 1024] -> heads [2048, 16, 64]
  k/v  = ctx_txt @ Wk/Wv          (77 text tokens)
  k2/v2= ctx_img @ Wk_ip/Wv_ip    (256 image tokens)
  out  = softmax(q k^T / 8) v + softmax(q k2^T / 8) v2
  y    = out @ Wo + bo

Device layout choices:
  - All matmul operands bf16 (fp32 psum accumulate). 4x faster PE than fp32.
  - Host pre-transposes x/context so activations arrive as xT [d_model, tok].
  - q is produced transposed (qT [inner, tok]); sim computed keys-on-partitions
    (simT [keys, tok]) so exp output feeds attn@v directly as the stationary
    operand; softmax denominator = extra ones-column appended to v.
  - attn@v output lands queries-on-partitions [128q, 65] so normalization is a
    per-partition scale; the 64-d head outputs are PE-transposed back to
    [inner, tok] for the output projection; bias added via per-partition
    tensor_scalar_add on the transposed output. Host un-transposes.
  - Next chunk's q-projection matmuls are interleaved into the attention head
    loop: independent GEMM work fills exp-wait bubbles and keeps the PE HAM
    clock gate warm.
"""

import numpy as np
import ml_dtypes

import concourse.bass as bass
import concourse.bacc as bacc
import concourse.tile as tile
import concourse.mybir as mybir
from concourse import bass_utils

BF16 = mybir.dt.bfloat16
F32 = mybir.dt.float32
NPBF16 = ml_dtypes.bfloat16

B, N, D = 16, 2048, 1024        # batch, query tokens, model dim (= inner dim)
H, DH = 16, 64                  # heads, head dim
LT, LI, L = 77, 256, 333        # text len, image len, total context len
NCORES = 8
BL = B // NCORES                # 2 batches per core
P = 128
TC = 512                        # query-token chunk (= one PSUM bank of fp32)
NCH = N // TC                   # 4 chunks per batch
NQS = TC // P                   # 4 query sub-tiles of 128 per chunk
SCALE = DH ** -0.5              # 0.125, folded into the exp() activation

TRACE = False
TMPDIR = None
LAST_RESULT = None
_NC = None


def _build_body(tc, io):
    nc = tc.nc
    import contextlib
    ctx = contextlib.ExitStack()
    with ctx:
        # ---------------- pools ----------------
        constp = ctx.enter_context(tc.tile_pool(name="const", bufs=1))
        wq = constp.tile([P, 8, D], BF16, name="wq")
        wo = constp.tile([P, 8, D], BF16, name="wo")
        ident = constp.tile([P, P], BF16, name="ident")
        bot = constp.tile([P, 8], F32, name="bot")

        kvp = ctx.enter_context(tc.tile_pool(name="kvout", bufs=1))
        kt_txt, kt_img, v_txt, v_img0, v_img1 = [], [], [], [], []
        for b in range(BL):
            kt_txt.append(kvp.tile([P, 8, LT], BF16, name=f"kttxt{b}"))
            kt_img.append(kvp.tile([P, 8, LI], BF16, name=f"ktimg{b}"))
            v_txt.append(kvp.tile([P, H * 65], BF16, name=f"vtxt{b}"))
            v_img0.append(kvp.tile([P, H * 65], BF16, name=f"vimg0{b}"))
            v_img1.append(kvp.tile([P, H * 65], BF16, name=f"vimg1{b}"))

        xqp = ctx.enter_context(tc.tile_pool(name="xq", bufs=2))
        bigp = ctx.enter_context(tc.tile_pool(name="big", bufs=2))
        smp = ctx.enter_context(tc.tile_pool(name="sm", bufs=3))
        # PSUM (8 banks): "sim" rotates 4 single-bank tiles (sim txt/img x2 and
        # chunk-end transposes; kv-phase accumulators too), "av" = txt+img
        # attn@v accumulators (2 banks), qp/op = q/out projection accumulators.
        gpsp = ctx.enter_context(tc.tile_pool(name="fxp", bufs=2, space="PSUM"))
        spsp = ctx.enter_context(tc.tile_pool(name="sps", bufs=4, space="PSUM"))
        avsp = ctx.enter_context(tc.tile_pool(name="avs", bufs=1, space="PSUM"))

        xTrs = [io["xT"].ap()[b].rearrange("(kc p) n -> p kc n", p=P)
                for b in range(BL)]
        oTrs = [io["outT"].ap()[b].rearrange("(oc p) n -> p oc n", p=P)
                for b in range(BL)]
        pairs = [(b, ch) for b in range(BL) for ch in range(NCH)]
        xts, qts = {}, {}

        def load_xt(b, ch):
            t = xqp.tile([P, 8, TC], BF16, tag="xt", name=f"xt{b}_{ch}")
            nc.sync.dma_start(t[:], xTrs[b][:, :, ch * TC:(ch + 1) * TC])
            xts[(b, ch)] = t

        def qproj_ic(b, ch, ic):
            """Emit one inner-chunk of the q projection for (b, ch)."""
            if (b, ch) not in qts:
                qts[(b, ch)] = bigp.tile(
                    [P, 8, TC], BF16, tag="qt", name=f"qt{b}_{ch}"
                )
            qt = qts[(b, ch)]
            xt = xts[(b, ch)]
            ps = gpsp.tile([P, TC], F32, tag="qp", bufs=1)
            for kc in range(8):
                nc.tensor.matmul(
                    ps[:], wq[:, kc, ic * P:(ic + 1) * P], xt[:, kc, :],
                    start=(kc == 0), stop=(kc == 7),
                )
            nc.vector.tensor_copy(qt[:, ic, :], ps[:])

        # ---------------- DMA issue order (single sync queue) -------------
        # First chunk's x + Wq (feeds the hoisted chunk-0 q projection), then
        # context + kv weights, then Wo (needed last).
        with tc.tile_pool(name="kvw", bufs=1) as kvwp:
            wk = kvwp.tile([P, 8, D], BF16, name="wk")
            wv = kvwp.tile([P, 8, D], BF16, name="wv")
            wkip = kvwp.tile([P, 8, D], BF16, name="wkip")
            wvip = kvwp.tile([P, 8, D], BF16, name="wvip")
            ctxTs = []
            for b in range(BL):
                ctxTs.append(
                    kvwp.tile([P, 8, L], BF16, tag="ctxT", name=f"ctxT{b}", bufs=2)
                )
                nc.sync.dma_start(
                    ctxTs[b][:], io["cT"].ap()[b].rearrange("(kc p) l -> p kc l", p=P)
                )
            for kc in range(8):
                for wt, nm in ((wk, "wk"), (wkip, "wkip")):
                    nc.sync.dma_start(
                        wt[:, kc, :], io[nm].ap()[kc * P:(kc + 1) * P, :]
                    )
            load_xt(0, 0)
            for kc in range(8):
                nc.sync.dma_start(
                    wq[:, kc, :], io["wq"].ap()[kc * P:(kc + 1) * P, :]
                )
            for kc in range(8):
                for wt, nm in ((wv, "wv"), (wvip, "wvip")):
                    nc.sync.dma_start(
                        wt[:, kc, :], io[nm].ap()[kc * P:(kc + 1) * P, :]
                    )
            nc.sync.dma_start(ident[:], io["ident"].ap())
            nc.sync.dma_start(bot[:], io["boT"].ap())
            for kc in range(8):
                nc.sync.dma_start(wo[:, kc, :], io["wo"].ap()[kc * P:(kc + 1) * P, :])
            load_xt(*pairs[1])

            # Hoisted chunk-0 q projection: PE works while kv weights stream.
            for ic in range(8):
                qproj_ic(0, 0, ic)

            # ---------------- phase 1: kv projections ----------------
            for b in range(BL):
                ctxT = ctxTs[b]
                # kT projections: out [inner-chunk, keys]
                for ic in range(8):
                    ps = spsp.tile([P, L], F32, tag="sim", name=f"kvk{b}_{ic}")
                    for kc in range(8):
                        nc.tensor.matmul(
                            ps[:, 0:LT],
                            wk[:, kc, ic * P:(ic + 1) * P],
                            ctxT[:, kc, 0:LT],
                            start=(kc == 0), stop=(kc == 7),
                        )
                    for kc in range(8):
                        nc.tensor.matmul(
                            ps[:, LT:L],
                            wkip[:, kc, ic * P:(ic + 1) * P],
                            ctxT[:, kc, LT:L],
                            start=(kc == 0), stop=(kc == 7),
                        )
                    nc.scalar.copy(kt_txt[b][:, ic, :], ps[:, 0:LT])
                    nc.scalar.copy(kt_img[b][:, ic, :], ps[:, LT:L])
                # v projections: out [keys, inner], evacuated into 65-col head
                # blocks (col 64 of each block later memset to 1.0 -> rowsums)
                for (vout, off, tl, wgt) in (
                    (v_txt[b], 0, LT, wv),
                    (v_img0[b], LT, P, wvip),
                    (v_img1[b], LT + P, P, wvip),
                ):
                    for nh in range(2):
                        ps2 = spsp.tile([P, TC], F32, tag="sim",
                                        name=f"kvv{b}_{off}_{nh}")
                        for kc in range(8):
                            nc.tensor.matmul(
                                ps2[0:tl, :],
                                ctxT[:, kc, off:off + tl],
                                wgt[:, kc, nh * TC:(nh + 1) * TC],
                                start=(kc == 0), stop=(kc == 7),
                            )
                        vview = vout[:].rearrange("p (h c) -> p h c", c=65)
                        psview = ps2[:].rearrange("p (h c) -> p h c", c=64)
                        nc.vector.tensor_copy(
                            vview[0:tl, nh * 8:(nh + 1) * 8, 0:64],
                            psview[0:tl, :, :],
                        )
                    vview = vout[:].rearrange("p (h c) -> p h c", c=65)
                    nc.vector.memset(vview[0:tl, :, 64:65], 1.0)

        # ---------------- phase 2: attention / out projection -------------
        attnTs, osbs = {}, {}

        def oproj_oc(b, ch, oc):
            """Emit one out-projection column-chunk for (b, ch)."""
            if (b, ch) not in osbs:
                osbs[(b, ch)] = bigp.tile(
                    [P, 8, TC], BF16, tag="osb", bufs=1, name=f"osb{b}_{ch}"
                )
            osb = osbs[(b, ch)]
            attnT = attnTs[(b, ch)]
            ps = gpsp.tile([P, TC], F32, tag="op", bufs=1)
            for ic in range(8):
                nc.tensor.matmul(
                    ps[:], wo[:, ic, oc * P:(oc + 1) * P], attnT[:, ic, :],
                    start=(ic == 0), stop=(ic == 7),
                )
            nc.vector.tensor_scalar_add(osb[:, oc, :], ps[:], bot[:, oc:oc + 1])

        def store_out(b, ch):
            attnTs.pop((b, ch))
            osb = osbs.pop((b, ch))
            nc.sync.dma_start(
                oTrs[b][:, :, ch * TC:(ch + 1) * TC], osb[:]
            )

        for pi, (b, chk) in enumerate(pairs):
            if pi + 2 < len(pairs):
                load_xt(*pairs[pi + 2])
            xts.pop((b, chk))
            qt = qts.pop((b, chk))
            nxt = pairs[pi + 1] if pi + 1 < len(pairs) else None
            prv = pairs[pi - 1] if pi > 0 else None

            attnA = bigp.tile([P, NQS, H, DH], BF16, tag="attnA")
            for h in range(H):
                # Interleave independent GEMM work between heads to fill
                # exp-wait bubbles on the PE and hide attn@v LDWEIGHTS:
                # odd heads -> next chunk's q projection, even heads ->
                # previous chunk's out projection.
                if nxt is not None and h % 2 == 1:
                    qproj_ic(nxt[0], nxt[1], h // 2)
                if prv is not None and h % 2 == 0:
                    oproj_oc(prv[0], prv[1], h // 2)
                ic, po = h // 2, (h % 2) * DH
                qh = qt[po:po + DH, ic, :]            # [64, TC]
                # simT = k q^T  (keys on partitions)
                pst = spsp.tile([P, TC], F32, tag="sim")
                nc.tensor.matmul(
                    pst[0:LT, :], kt_txt[b][po:po + DH, ic, :], qh,
                    start=True, stop=True,
                )
                et = smp.tile([P, TC], BF16, tag="et")
                nc.scalar.activation(
                    et[0:LT, :], pst[0:LT, :],
                    mybir.ActivationFunctionType.Exp, scale=SCALE,
                )
                ei = smp.tile([P, 2, TC], BF16, tag="ei")
                for k2 in range(2):
                    psi = spsp.tile([P, TC], F32, tag="sim")
                    nc.tensor.matmul(
                        psi[:],
                        kt_img[b][po:po + DH, ic, k2 * P:(k2 + 1) * P],
                        qh,
                        start=True, stop=True,
                    )
                    nc.scalar.activation(
                        ei[:, k2, :], psi[:],
                        mybir.ActivationFunctionType.Exp, scale=SCALE,
                    )
                # attn @ [v | 1]: out [128q, 65] per query sub-tile;
                # txt group in bank 0, img group in bank 1
                av = avsp.tile([P, 2, TC], F32, tag="av")
                for qs in range(NQS):
                    csl = slice(qs * 65, qs * 65 + 65)
                    nc.tensor.matmul(
                        av[:, 0, csl],
                        et[0:LT, qs * P:(qs + 1) * P],
                        v_txt[b][0:LT, h * 65:(h + 1) * 65],
                        start=True, stop=True,
                    )
                    nc.tensor.matmul(
                        av[:, 1, csl],
                        ei[:, 0, qs * P:(qs + 1) * P],
                        v_img0[b][:, h * 65:(h + 1) * 65],
                        start=True, stop=False,
                    )
                    nc.tensor.matmul(
                        av[:, 1, csl],
                        ei[:, 1, qs * P:(qs + 1) * P],
                        v_img1[b][:, h * 65:(h + 1) * 65],
                        start=False, stop=True,
                    )
                # normalize (per-partition = per-query) and sum txt+img
                avg = av[:, :, 0:NQS * 65].rearrange(
                    "p g (q c) -> p g q c", c=65
                )
                r2 = smp.tile([P, 2, NQS], F32, tag="r2")
                tt = smp.tile([P, 2, NQS, DH], BF16, tag="tt")
                nc.vector.reciprocal(r2[:], avg[:, :, :, 64])
                nc.vector.tensor_tensor(
                    tt[:],
                    avg[:, :, :, 0:DH],
                    r2[:].to_broadcast((P, 2, NQS, DH)),
                    op=mybir.AluOpType.mult,
                )
                nc.vector.tensor_add(attnA[:, :, h, :], tt[:, 0], tt[:, 1])

            # previous chunk fully projected by now: store it
            if prv is not None:
                store_out(*prv)

            # transpose attn back to [inner, tok] in 128x128 blocks
            attnT = bigp.tile([P, 8, TC], BF16, tag="attnT", bufs=1,
                              name=f"attnT{b}_{chk}")
            attnTs[(b, chk)] = attnT
            for qs in range(NQS):
                for hg in range(2):
                    pt = spsp.tile([P, 4, P], BF16, tag="sim")
                    for j in range(4):
                        hc = hg * 4 + j
                        nc.tensor.transpose(
                            pt[:, j, :],
                            attnA[:, qs, 2 * hc:2 * hc + 2, :],
                            ident[:],
                        )
                    nc.vector.tensor_copy(
                        attnT[:, hg * 4:(hg + 1) * 4, qs * P:(qs + 1) * P],
                        pt[:],
                    )

        # tail: out projection + store of the final chunk
        last = pairs[-1]
        for oc in range(8):
            oproj_oc(last[0], last[1], oc)
        store_out(*last)


def build():
    global _NC
    if _NC is not None:
        return _NC
    nc = bacc.Bacc("TRN2", target_bir_lowering=False, debug=False,
                   num_devices=NCORES)
    io = {
        "xT": nc.dram_tensor("xT", [BL, D, N], BF16, kind="ExternalInput"),
        "cT": nc.dram_tensor("cT", [BL, D, L], BF16, kind="ExternalInput"),
        "wq": nc.dram_tensor("wq", [D, D], BF16, kind="ExternalInput"),
        "wk": nc.dram_tensor("wk", [D, D], BF16, kind="ExternalInput"),
        "wv": nc.dram_tensor("wv", [D, D], BF16, kind="ExternalInput"),
        "wkip": nc.dram_tensor("wkip", [D, D], BF16, kind="ExternalInput"),
        "wvip": nc.dram_tensor("wvip", [D, D], BF16, kind="ExternalInput"),
        "wo": nc.dram_tensor("wo", [D, D], BF16, kind="ExternalInput"),
        "boT": nc.dram_tensor("boT", [P, 8], F32, kind="ExternalInput"),
        "ident": nc.dram_tensor("ident", [P, P], BF16, kind="ExternalInput"),
        "outT": nc.dram_tensor("outT", [BL, D, N], BF16, kind="ExternalOutput"),
    }
    with tile.TileContext(nc) as tc:
        _build_body(tc, io)
    nc.compile()
    _NC = nc
    return nc


def kernel(x, context, Wq, Wk, Wv, Wk_ip, Wv_ip, Wo, bo):
    global LAST_RESULT
    nc = build()

    xT = np.ascontiguousarray(x.astype(NPBF16).transpose(0, 2, 1))
    cT = np.ascontiguousarray(context.astype(NPBF16).transpose(0, 2, 1))
    shared = {
        "wq": np.ascontiguousarray(Wq.astype(NPBF16)),
        "wk": np.ascontiguousarray(Wk.astype(NPBF16)),
        "wv": np.ascontiguousarray(Wv.astype(NPBF16)),
        "wkip": np.ascontiguousarray(Wk_ip.astype(NPBF16)),
        "wvip": np.ascontiguousarray(Wv_ip.astype(NPBF16)),
        "wo": np.ascontiguousarray(Wo.astype(NPBF16)),
        "boT": np.ascontiguousarray(bo.astype(np.float32).reshape(8, P).T),
        "ident": np.eye(P, dtype=NPBF16),
    }
    in_maps = [
        {"xT": xT[c * BL:(c + 1) * BL], "cT": cT[c * BL:(c + 1) * BL], **shared}
        for c in range(NCORES)
    ]
    res = bass_utils.run_bass_kernel_spmd(
        nc, in_maps, core_ids=list(range(NCORES)), trace=TRACE, tmpdir=TMPDIR
    )
    LAST_RESULT = res
    out = np.concatenate(
        [r["outT"].transpose(0, 2, 1).astype(np.float32) for r in res.results],
        axis=0,
    )
    return np.ascontiguousarray(out)
